# revision 1
# baseline (speedup 1.0000x reference)
"""Trainium2 Bass kernel for nn_CrossAttention_7584912245418.

Sharding: batch*head-blocks across 8 cores. Core c handles batch b=c//2 and
head block hb=c%2 (8 of 16 heads). Weights column/row-sliced per head block;
no cross-device communication. Host feeds pre-transposed bf16 activations
(xT, cT) so every on-chip matmul contracts over the partition dim, and sums
the two per-batch partial outputs (Wo row-split) + bias at the end.

Per-core pipeline (all layouts transposed, d-on-partitions):
  KT = Wk_s^T @ cT                          [512, NK] bf16
  V  = cT^T @ Wv_s -> V_aug [j, h*65+e]     (e=0..63 V, e=64 mask col; rows
                                             scaled by mask -> free masking +
                                             softmax denominator via matmul)
  per head pair p (interleaved into the attention stream):
     QT_p = Wq_s^T @ xT                     projection overlapped with exp
  per head: S^T = KT_h-chunks.T @ QT_h      (K=64 matmuls, PSUM f32)
            P^T = exp(SCALE*S^T)            (ACT, scale fused, no max-sub:
                                             scores bounded ~+-3 by input dist)
            O^T_aug = V_aug_h.T @ P^T       (accumulate over j in PSUM,
                                             row 64 = masked rowsum)
            OT = (O^T / rowsum)             (DVE recip + gpsimd bcast + mul)
  out_partial = OT_all-chunks.T @ Wo_s      -> [NQ, 1024] f32

PV of unit u-1 interleaves with QK/exp of unit u so PE stays busy while ACT
(the exp throughput bound, ~267us) churns; Q-projection of pair p+1 rides in
the same window.
"""

import sys

for _p in ("/opt/trn_rl_repo",):
    if _p not in sys.path:
        sys.path.insert(0, _p)

from contextlib import ExitStack

import ml_dtypes
import numpy as np

import concourse.bass as bass
import concourse.mybir as mybir
import concourse.tile as tile
from concourse import bacc
from concourse.bass_utils import run_bass_kernel_spmd

F32 = mybir.dt.float32
BF16 = mybir.dt.bfloat16
AF = mybir.ActivationFunctionType

# Full-problem constants
B, N, M = 4, 2048, 2048
QD, CD, OD = 1024, 1024, 1024
H, DH = 16, 64
SCALE = DH ** -0.5
NCORES = 8
NH = 8            # heads per core
HD = NH * DH      # 512, per-core inner dim
P = 128


def build_nc(NQ=N, NK=M, KD=QD, trace_sim=False):
    """Build the per-core SPMD program. NQ=query len, NK=kv len, KD=model dim."""
    KC = KD // P          # contraction chunks for projections
    JC = NK // P          # key-position chunks
    IC = NQ // P          # query-position chunks
    SP = 512              # matmul free-dim span
    NSP = NQ // SP        # spans over queries
    IH = min(1024, NQ)    # exp granularity (free elems per ACT instr)
    NIH = NQ // IH
    SPI = IH // SP        # spans per exp block
    DC = HD // P          # 4 head-pair chunks (2 heads per chunk)
    ODS = OD // SP        # output spans
    VW = NH * 65          # v_aug row width per j-chunk

    nc = bacc.Bacc("TRN2", target_bir_lowering=False, debug=False,
                   enable_asserts=False)

    xt_d = nc.dram_tensor("xt", [KD, NQ], BF16, kind="ExternalInput")
    ct_d = nc.dram_tensor("ct", [KD, NK], BF16, kind="ExternalInput")
    wq_d = nc.dram_tensor("wq", [KD, HD], BF16, kind="ExternalInput")
    wk_d = nc.dram_tensor("wk", [KD, HD], BF16, kind="ExternalInput")
    wv_d = nc.dram_tensor("wv", [KD, HD], BF16, kind="ExternalInput")
    wo_d = nc.dram_tensor("wo", [HD, OD], BF16, kind="ExternalInput")
    mk_d = nc.dram_tensor("mk", [NK], F32, kind="ExternalInput")
    out_d = nc.dram_tensor("out", [NQ, OD], F32, kind="ExternalOutput")

    with tile.TileContext(nc, trace_sim=trace_sim) as tc, ExitStack() as ctx:
        # ---- persistent pools ----
        pp = ctx.enter_context(tc.tile_pool(name="persist", bufs=1))
        qt = pp.tile([P, DC * NQ], BF16, tag="qt")
        kt = pp.tile([P, DC * NK], BF16, tag="kt")
        vaug = pp.tile([P, JC * VW], BF16, tag="vaug")
        mkt = pp.tile([P, JC], F32, tag="mkt")
        ot_all = pp.tile([P, DC * NQ], BF16, tag="ot_all")

        ps_pool = ctx.enter_context(tc.tile_pool(name="ps", bufs=2, space="PSUM"))
        po_pool = ctx.enter_context(tc.tile_pool(name="po", bufs=1, space="PSUM"))

        # xt/wq live until the last Q projection (inside the attention loop)
        xq_stack = ExitStack()
        xq = xq_stack.enter_context(tc.tile_pool(name="xq", bufs=1, side="right"))
        xts = [xq.tile([P, NQ], BF16, tag=f"xt{k}", name=f"xt{k}")
               for k in range(KC)]
        wqs = [xq.tile([P, HD], BF16, tag=f"wq{k}", name=f"wq{k}")
               for k in range(KC)]

        pt_pool = ctx.enter_context(tc.tile_pool(name="pt", bufs=2))
        pts = {}
        po_tiles = {}

        # ---- load + K projection (ct/wk/wv scoped; Vproj after unit 0) ----
        ip_stack = ExitStack()
        ip = ip_stack.enter_context(tc.tile_pool(name="inp", bufs=1))
        if True:
            cts = [ip.tile([P, NK], BF16, tag=f"ct{k}", name=f"ct{k}")
                   for k in range(KC)]
            wks = [ip.tile([P, HD], BF16, tag=f"wk{k}", name=f"wk{k}")
                   for k in range(KC)]
            wvs = [ip.tile([P, HD], BF16, tag=f"wv{k}", name=f"wv{k}")
                   for k in range(KC)]

            nc.sync.dma_start(mkt[:], mk_d.ap().rearrange("(jc p) -> p jc", p=P))
            for k in range(KC):
                r = slice(k * P, (k + 1) * P)
                nc.sync.dma_start(wks[k][:], wk_d.ap()[r, :])
                nc.sync.dma_start(cts[k][:], ct_d.ap()[r, :])
            for k in range(KC):
                r = slice(k * P, (k + 1) * P)
                nc.sync.dma_start(wvs[k][:], wv_d.ap()[r, :])
                nc.sync.dma_start(wqs[k][:], wq_d.ap()[r, :])
                nc.sync.dma_start(xts[k][:], xt_d.ap()[r, :])

            # ones into v_aug mask columns, scaled by mask per j-chunk below
            vz = vaug[:].rearrange("p (jc h e) -> p (jc h) e", jc=JC, h=NH)
            nc.vector.memset(vz[:, :, 64:65], 1.0)

            # ---- K projection: KT[d, j] ----
            for d in range(DC):
                for s in range(NK // SP):
                    ps = ps_pool.tile([P, SP], F32, tag="ps")
                    for k in range(KC):
                        nc.tensor.matmul(
                            ps[:], wks[k][:, d * P:(d + 1) * P],
                            cts[k][:, s * SP:(s + 1) * SP],
                            start=(k == 0), stop=(k == KC - 1))
                    nc.vector.tensor_copy(
                        kt[:, d * NK + s * SP: d * NK + (s + 1) * SP], ps[:])

        def emit_vproj():
            # V projection -> v_aug (mask-scaled)
            for j in range(JC):
                ps = ps_pool.tile([P, HD], F32, tag="ps")
                for k in range(KC):
                    nc.tensor.matmul(
                        ps[:], cts[k][:, j * P:(j + 1) * P], wvs[k][:],
                        start=(k == 0), stop=(k == KC - 1))
                dst = vaug[:, j * VW:(j + 1) * VW].rearrange(
                    "p (h e) -> p h e", h=NH)
                nc.vector.tensor_scalar_mul(
                    dst[:, :, 0:64], ps[:].rearrange("p (h e) -> p h e", h=NH),
                    mkt[:, j:j + 1])
                nc.vector.tensor_scalar_mul(
                    dst[:, :, 64:65], dst[:, :, 64:65], mkt[:, j:j + 1])

        def emit_qproj(d):
            for s in range(NSP):
                ps = ps_pool.tile([P, SP], F32, tag="ps")
                for k in range(KC):
                    nc.tensor.matmul(
                        ps[:], wqs[k][:, d * P:(d + 1) * P],
                        xts[k][:, s * SP:(s + 1) * SP],
                        start=(k == 0), stop=(k == KC - 1))
                nc.vector.tensor_copy(
                    qt[:, d * NQ + s * SP: d * NQ + (s + 1) * SP], ps[:])

        # ---- attention: units = (head, j-quarter); PV of unit u-1
        # interleaves with QK/exp of unit u; Qproj of pair p rides along ----
        JH = max(1, JC // 4)
        upj = JC // JH                # units per head
        units = []
        for h in range(NH):
            for q in range(upj):
                units.append((h, [q * JH + j for j in range(JH)]))

        drain_pool = None

        def emit_qk(u, idx):
            h, jcs = units[u]
            jc = jcs[idx]
            dc, hoff = h // 2, (h % 2) * 64
            for b in range(NIH):
                ps = ps_pool.tile([P, IH], F32, tag="ps")
                for s in range(SPI):
                    i0 = b * IH + s * SP
                    nc.tensor.matmul(
                        ps[:, s * SP:(s + 1) * SP],
                        kt[hoff:hoff + 64, dc * NK + jc * P: dc * NK + (jc + 1) * P],
                        qt[hoff:hoff + 64, dc * NQ + i0: dc * NQ + i0 + SP],
                        start=True, stop=True)
                nc.scalar.activation(
                    pts[u][:, idx * NQ + b * IH: idx * NQ + (b + 1) * IH],
                    ps[:], AF.Exp, scale=SCALE)

        def emit_pv(u, idx):
            h, jcs = units[u]
            jc = jcs[idx]
            first = (u % upj == 0) and idx == 0
            last = (u % upj == upj - 1) and idx == len(jcs) - 1
            po = po_tiles[h]
            for s in range(NSP):
                nc.tensor.matmul(
                    po[0:65, s * SP:(s + 1) * SP],
                    vaug[:, jc * VW + h * 65: jc * VW + (h + 1) * 65],
                    pts[u][:, idx * NQ + s * SP: idx * NQ + (s + 1) * SP],
                    start=first, stop=last)

        def emit_norm(h):
            dc, hoff = h // 2, (h % 2) * 64
            po = po_tiles.pop(h)
            ot_un = drain_pool.tile([65, NQ], F32, tag="ot_un")
            nc.vector.tensor_copy(ot_un[:], po[0:65, :])
            rinv = drain_pool.tile([1, NQ], F32, tag="rinv")
            nc.sync.dma_start(rinv[:], ot_un[64:65, :])
            nc.vector.reciprocal(rinv[:], rinv[:])
            rb = drain_pool.tile([64, NQ], F32, tag="rb")
            nc.gpsimd.partition_broadcast(rb[:], rinv[:])
            ot_n = drain_pool.tile([64, NQ], BF16, tag="ot_n")
            nc.vector.tensor_mul(ot_n[:], ot_un[0:64, :], rb[:])
            nc.sync.dma_start(
                ot_all[hoff:hoff + 64, dc * NQ:(dc + 1) * NQ], ot_n[:])

        nunits = len(units)
        for u in range(nunits):
            h = units[u][0]
            if u == 0:
                emit_qproj(0)
            # prefetch next pair's Q projection mid-pair so the pair
            # boundary has no PE bubble in front of the first QK
            if (u + upj) % (2 * upj) == 0 and (u + upj) < nunits:
                emit_qproj((u + upj) // (2 * upj))
            if u % upj == 0:
                po_tiles[h] = po_pool.tile([P, NQ], F32, tag="po",
                                           name=f"po{h}")
            pts[u] = pt_pool.tile([P, JH * NQ], BF16, tag="pt", name=f"pt{u}")
            for idx in range(JH):
                emit_qk(u, idx)
                if u > 0:
                    emit_pv(u - 1, idx)
            if u == 0:
                emit_vproj()              # V rides in pair-0's exp window
                ip_stack.close()
                drain_pool = ctx.enter_context(
                    tc.tile_pool(name="drain", bufs=1))
            if u > 0 and (u - 1) % upj == upj - 1:
                emit_norm(units[u - 1][0])
            pts.pop(u - 2, None)
        for idx in range(JH):
            emit_pv(nunits - 1, idx)
        emit_norm(units[nunits - 1][0])
        xq_stack.close()

        out_pool = ctx.enter_context(tc.tile_pool(name="outp", bufs=2))
        wo_t = out_pool.tile([P, DC * OD], BF16, tag="wo_t", bufs=1)
        for d in range(DC):
            nc.sync.dma_start(wo_t[:, d * OD:(d + 1) * OD],
                              wo_d.ap()[d * P:(d + 1) * P, :])

        # ---- output projection ----
        for i in range(IC):
            osb = out_pool.tile([P, OD], F32, tag="osb")
            for o in range(ODS):
                ps = ps_pool.tile([P, SP], F32, tag="ps")
                for d in range(DC):
                    nc.tensor.matmul(
                        ps[:],
                        ot_all[:, d * NQ + i * P: d * NQ + (i + 1) * P],
                        wo_t[:, d * OD + o * SP: d * OD + (o + 1) * SP],
                        start=(d == 0), stop=(d == DC - 1))
                nc.vector.tensor_copy(osb[:, o * SP:(o + 1) * SP], ps[:])
            nc.sync.dma_start(out_d.ap()[i * P:(i + 1) * P, :], osb[:])

    nc.compile()
    return nc


def shard_inputs(x, context, mask, Wq, Wk, Wv, Wo):
    """Host-side shard prep: per-core bf16 transposed inputs."""
    bf = ml_dtypes.bfloat16
    in_maps = []
    for c in range(NCORES):
        b, hb = c // 2, c % 2
        cols = slice(hb * HD, (hb + 1) * HD)
        in_maps.append({
            "xt": np.ascontiguousarray(x[b].T).astype(bf),
            "ct": np.ascontiguousarray(context[b].T).astype(bf),
            "wq": np.ascontiguousarray(Wq[:, cols]).astype(bf),
            "wk": np.ascontiguousarray(Wk[:, cols]).astype(bf),
            "wv": np.ascontiguousarray(Wv[:, cols]).astype(bf),
            "wo": np.ascontiguousarray(Wo[cols, :]).astype(bf),
            "mk": mask[b].astype(np.float32),
        })
    return in_maps


_NC_CACHE = {}


def kernel(x, context, mask, Wq, Wk, Wv, Wo, bo, _trace=False):
    x = np.asarray(x, np.float32)
    context = np.asarray(context, np.float32)
    mask = np.asarray(mask)
    Wq, Wk, Wv = (np.asarray(a, np.float32) for a in (Wq, Wk, Wv))
    Wo, bo = np.asarray(Wo, np.float32), np.asarray(bo, np.float32)

    if "nc" not in _NC_CACHE:
        _NC_CACHE["nc"] = build_nc()
    nc = _NC_CACHE["nc"]

    in_maps = shard_inputs(x, context, mask, Wq, Wk, Wv, Wo)
    res = run_bass_kernel_spmd(nc, in_maps, list(range(NCORES)), trace=_trace)
    out = np.zeros((B, N, OD), np.float32)
    for c in range(NCORES):
        out[c // 2] += res.results[c]["out"]
    out += bo
    _NC_CACHE["last_res"] = res
    return out



# revision 22
# speedup vs baseline: 1.2228x; 1.2228x over previous
"""Trainium2 Bass kernel for nn_CrossAttention_7584912245418.

Sharding: batch*head-blocks across 8 cores. Core c handles batch b=c//2 and
head block hb=c%2 (8 of 16 heads). Weights column/row-sliced per head block;
no cross-device communication. Host feeds pre-transposed bf16 activations
(xT, cT); the two per-batch partial outputs (Wo row-split) + bias are summed
on host.

Per-core pipeline (cost-model-aware layout):
  KT = Wk_s^T @ cT                      [d, j] bf16 (k-streamed vs ct DMA)
  QT = Wq_s^T @ xT                      [d, i] bf16 (per d-chunk, per i-half)
  V  = cT^T @ Wv_s -> vaug[j, h, 0:64]  mask-scaled; col 64 = mask (rowsum)
  units = (i-half, head):
    per jc: S^T[j,i] = KT_chunk^T @ QT      (PSUM f32 [128, IHALF])
            P^T = exp(SCALE*S^T)            (ACT, bf16, no max-sub)
            O[i,d] += P^T_chunk^T @ V_chunk (tiny matmuls: 65 rows each ->
                                             2x cheaper than streaming NQ)
            rowsum[i] += P^T_chunk^T @ mask col
    norm: per-partition recip+mul (no gpsimd broadcast), PE transpose back
          to [d, i], store bf16 ot_all.
  out = OT^T @ Wo per i-chunk; i-half 0's output projection hides inside
  i-half 1's exp window (ACT-bound), as do Q/K-proj leftovers.
"""

import sys

for _p in ("/opt/trn_rl_repo",):
    if _p not in sys.path:
        sys.path.insert(0, _p)

from contextlib import ExitStack

import ml_dtypes
import numpy as np

import concourse.bass as bass
import concourse.mybir as mybir
import concourse.tile as tile
from concourse import bacc
from concourse.bass_utils import run_bass_kernel_spmd
from concourse.masks import make_identity

F32 = mybir.dt.float32
BF16 = mybir.dt.bfloat16
AF = mybir.ActivationFunctionType

# Full-problem constants
B, N, M = 4, 2048, 2048
QD, CD, OD = 1024, 1024, 1024
H, DH = 16, 64
SCALE = DH ** -0.5
NCORES = 8
NH = 8            # heads per core
HD = NH * DH      # 512, per-core inner dim
P = 128


def build_nc(NQ=N, NK=M, KD=QD, trace_sim=False):
    """Build the per-core SPMD program. NQ=query len, NK=kv len, KD=model dim."""
    KC = KD // P            # contraction chunks for projections
    JC = NK // P            # key-position chunks
    IC = NQ // P            # query-position chunks
    SP = 512                # projection matmul free-dim span
    IHALF = min(1024, NQ)   # queries per attention unit (and exp granularity)
    NIH = NQ // IHALF
    ISP = IHALF // SP       # spans per i-half
    ICH = IHALF // P        # i-chunks per half
    DC = HD // P            # 4 head-pair chunks (2 heads per chunk)
    VW = NH * 65            # vaug row width per j-chunk
    NUNITS = NIH * NH

    nc = bacc.Bacc("TRN2", target_bir_lowering=False, debug=False,
                   enable_asserts=False)

    xt_d = nc.dram_tensor("xt", [KD, NQ], BF16, kind="ExternalInput")
    ct_d = nc.dram_tensor("ct", [KD, NK], BF16, kind="ExternalInput")
    wq_d = nc.dram_tensor("wq", [KD, HD], BF16, kind="ExternalInput")
    wk_d = nc.dram_tensor("wk", [KD, HD], BF16, kind="ExternalInput")
    wv_d = nc.dram_tensor("wv", [KD, HD], BF16, kind="ExternalInput")
    wo_d = nc.dram_tensor("wo", [HD, OD], BF16, kind="ExternalInput")
    mk_d = nc.dram_tensor("mk", [NK], F32, kind="ExternalInput")
    out_d = nc.dram_tensor("out", [NQ, OD], F32, kind="ExternalOutput")

    with tile.TileContext(nc, trace_sim=trace_sim) as tc, ExitStack() as ctx:
        # ---- persistent SBUF ----
        pp = ctx.enter_context(tc.tile_pool(name="persist", bufs=1))
        kt = pp.tile([P, DC * NK], BF16, tag="kt")
        qt = pp.tile([P, DC * NQ], BF16, tag="qt")
        vaug = pp.tile([P, JC * VW], BF16, tag="vaug")
        mkt = pp.tile([P, JC], F32, tag="mkt")
        ot_all = pp.tile([P, DC * NQ], BF16, tag="ot_all")
        ident = pp.tile([P, P], BF16, tag="ident")
        wo_t = pp.tile([P, DC * OD], BF16, tag="wo_t")
        cts = [pp.tile([P, NK], BF16, tag=f"ct{k}", name=f"ct{k}")
               for k in range(KC)]
        wks = [pp.tile([P, HD], BF16, tag=f"wk{k}", name=f"wk{k}")
               for k in range(KC)]
        wvs = [pp.tile([P, HD], BF16, tag=f"wv{k}", name=f"wv{k}")
               for k in range(KC)]
        xts = [pp.tile([P, NQ], BF16, tag=f"xt{k}", name=f"xt{k}")
               for k in range(KC)]
        wqs = [pp.tile([P, HD], BF16, tag=f"wq{k}", name=f"wq{k}")
               for k in range(KC)]

        pts_pool = ctx.enter_context(tc.tile_pool(name="pts", bufs=12))
        on_pool = ctx.enter_context(tc.tile_pool(name="on", bufs=2))
        rv_pool = ctx.enter_context(tc.tile_pool(name="rv", bufs=2))
        osb_pool = ctx.enter_context(tc.tile_pool(name="osb", bufs=3))

        # ---- input DMAs (order = DMA device order: K-proj inputs first) ----
        nc.sync.dma_start(mkt[:], mk_d.ap().rearrange("(jc p) -> p jc", p=P))
        for k in range(KC):
            r = slice(k * P, (k + 1) * P)
            nc.sync.dma_start(wks[k][:], wk_d.ap()[r, :])
            nc.sync.dma_start(cts[k][:], ct_d.ap()[r, :])
        for k in range(KC):
            r = slice(k * P, (k + 1) * P)
            nc.sync.dma_start(wvs[k][:], wv_d.ap()[r, :])
        for k in range(KC):
            r = slice(k * P, (k + 1) * P)
            nc.sync.dma_start(wqs[k][:, 0:P], wq_d.ap()[r, 0:P])
            nc.sync.dma_start(xts[k][:, 0:IHALF], xt_d.ap()[r, 0:IHALF])
        for k in range(KC):
            r = slice(k * P, (k + 1) * P)
            nc.sync.dma_start(wqs[k][:, P:HD], wq_d.ap()[r, P:HD])
            if NQ > IHALF:
                nc.sync.dma_start(xts[k][:, IHALF:NQ], xt_d.ap()[r, IHALF:NQ])
        for d in range(DC):
            nc.sync.dma_start(wo_t[:, d * OD:(d + 1) * OD],
                              wo_d.ap()[d * P:(d + 1) * P, :])

        make_identity(nc, ident[:])
        vz = vaug[:].rearrange("p (jc h e) -> p (jc h) e", jc=JC, h=NH)
        nc.vector.memset(vz[:, :, 64:65], 1.0)

        # ---- K-proj d=0,1 k-streamed against the ct DMA; then Q(d0, ih0) ----
        NSPK = NK // SP
        with tc.tile_pool(name="kp", bufs=2 * NSPK, space="PSUM") as kp:
            ka = [kp.tile([P, SP], F32, tag="ka", name=f"ka{t}")
                  for t in range(2 * NSPK)]
            for k in range(KC):
                for d in range(2):
                    for s in range(NSPK):
                        nc.tensor.matmul(
                            ka[d * NSPK + s][:], wks[k][:, d * P:(d + 1) * P],
                            cts[k][:, s * SP:(s + 1) * SP],
                            start=(k == 0), stop=(k == KC - 1))
            for d in range(2):
                for s in range(NSPK):
                    nc.vector.tensor_copy(
                        kt[:, d * NK + s * SP: d * NK + (s + 1) * SP],
                        ka[d * NSPK + s][:])
            for jc in range(JC // 4):
                vp = kp.tile([P, HD], F32, tag="ka", name=f"vpk{jc}")
                for k in range(KC):
                    nc.tensor.matmul(
                        vp[:], cts[k][:, jc * P:(jc + 1) * P], wvs[k][:],
                        start=(k == 0), stop=(k == KC - 1))
                dstv = vaug[:, jc * VW:(jc + 1) * VW].rearrange(
                    "p (h e) -> p h e", h=NH)
                nc.vector.tensor_scalar_mul(
                    dstv[:, :, 0:64],
                    vp[:].rearrange("p (h e) -> p h e", h=NH),
                    mkt[:, jc:jc + 1])
                nc.vector.tensor_scalar_mul(
                    dstv[:, :, 64:65], dstv[:, :, 64:65], mkt[:, jc:jc + 1])
            for s in range(ISP):
                q0 = kp.tile([P, SP], F32, tag="ka", name=f"q0_{s}")
                for k in range(KC):
                    nc.tensor.matmul(
                        q0[:], wqs[k][:, 0:P], xts[k][:, s * SP:(s + 1) * SP],
                        start=(k == 0), stop=(k == KC - 1))
                nc.vector.tensor_copy(qt[:, s * SP:(s + 1) * SP], q0[:])

        # ---- attention-phase PSUM pools ----
        ps_pool = ctx.enter_context(
            tc.tile_pool(name="ps", bufs=2, space="PSUM"))
        po_pool = ctx.enter_context(
            tc.tile_pool(name="po", bufs=2, space="PSUM"))
        prp = ctx.enter_context(tc.tile_pool(name="pr", bufs=1, space="PSUM"))
        pr = prp.tile([P, ICH], F32, tag="pr")
        misc_pool = ctx.enter_context(
            tc.tile_pool(name="misc", bufs=1, space="PSUM"))

        # ---- filler emitters (PE work to hide under the exp pipeline) ----
        # Span tasks split into 2 pieces of KC/2 matmuls each so a filler
        # burst never outruns ACT's short exp pipeline buffer.
        def _pieces(alloc, mm, fin):
            st = {}

            def p1():
                st["t"] = alloc()
                mm(st["t"], 0, KC // 2)

            def p2():
                mm(st["t"], KC // 2, KC)
                fin(st["t"])
            return [p1, p2]

        def kproj_pieces(d, s0):
            def alloc():
                return misc_pool.tile([P, SP], F32, tag="m", name=f"kp{d}_{s0}")

            def mm(t, k0, k1):
                for k in range(k0, k1):
                    nc.tensor.matmul(
                        t[:], wks[k][:, d * P:(d + 1) * P],
                        cts[k][:, s0 * SP:(s0 + 1) * SP],
                        start=(k == 0), stop=(k == KC - 1))

            def fin(t):
                nc.vector.tensor_copy(
                    kt[:, d * NK + s0 * SP: d * NK + (s0 + 1) * SP], t[:])
            return _pieces(alloc, mm, fin)

        def qproj_pieces(d, ih, sq):
            c0 = ih * IHALF + sq * SP

            def alloc():
                return misc_pool.tile([P, SP], F32, tag="m",
                                      name=f"qp{d}_{ih}_{sq}")

            def mm(t, k0, k1):
                for k in range(k0, k1):
                    nc.tensor.matmul(
                        t[:], wqs[k][:, d * P:(d + 1) * P],
                        xts[k][:, c0:c0 + SP],
                        start=(k == 0), stop=(k == KC - 1))

            def fin(t):
                nc.vector.tensor_copy(
                    qt[:, d * NQ + c0: d * NQ + c0 + SP], t[:])
            return _pieces(alloc, mm, fin)

        def vproj_into(pool, tag, jc):
            """V projection for one j-chunk into vaug via a [P, HD] psum tile."""
            ps = pool.tile([P, HD], F32, tag=tag, name=f"vp{jc}")
            for k in range(KC):
                nc.tensor.matmul(
                    ps[:], cts[k][:, jc * P:(jc + 1) * P], wvs[k][:],
                    start=(k == 0), stop=(k == KC - 1))
            dst = vaug[:, jc * VW:(jc + 1) * VW].rearrange(
                "p (h e) -> p h e", h=NH)
            nc.vector.tensor_scalar_mul(
                dst[:, :, 0:64],
                ps[:].rearrange("p (h e) -> p h e", h=NH),
                mkt[:, jc:jc + 1])
            nc.vector.tensor_scalar_mul(
                dst[:, :, 64:65], dst[:, :, 64:65], mkt[:, jc:jc + 1])

        def emit_oproj(i, tail=False):
            """Output projection for global i-chunk i."""
            if tail:
                op = ps_pool.tile([P, OD], F32, tag="ps", name=f"op{i}")
                for o in range(OD // SP):
                    for d in range(DC):
                        nc.tensor.matmul(
                            op[:, o * SP:(o + 1) * SP],
                            ot_all[:, d * NQ + i * P: d * NQ + (i + 1) * P],
                            wo_t[:, d * OD + o * SP: d * OD + (o + 1) * SP],
                            start=(d == 0), stop=(d == DC - 1))
                osb = osb_pool.tile([P, OD], F32, tag="osb")
                nc.vector.tensor_copy(osb[:], op[:])
                nc.sync.dma_start(out_d.ap()[i * P:(i + 1) * P, :], osb[:])
                return
            osb = osb_pool.tile([P, OD], F32, tag="osb")
            if True:
                for o in range(OD // SP):
                    op = misc_pool.tile([P, SP], F32, tag="m",
                                        name=f"op{i}_{o}")
                    for d in range(DC):
                        nc.tensor.matmul(
                            op[:],
                            ot_all[:, d * NQ + i * P: d * NQ + (i + 1) * P],
                            wo_t[:, d * OD + o * SP: d * OD + (o + 1) * SP],
                            start=(d == 0), stop=(d == DC - 1))
                    nc.vector.tensor_copy(osb[:, o * SP:(o + 1) * SP], op[:])
            nc.sync.dma_start(out_d.ap()[i * P:(i + 1) * P, :], osb[:])

        def oproj_span(i, o, osb):
            op = misc_pool.tile([P, SP], F32, tag="m", name=f"op{i}_{o}")
            for d in range(DC):
                nc.tensor.matmul(
                    op[:], ot_all[:, d * NQ + i * P: d * NQ + (i + 1) * P],
                    wo_t[:, d * OD + o * SP: d * OD + (o + 1) * SP],
                    start=(d == 0), stop=(d == DC - 1))
            nc.vector.tensor_copy(osb[:, o * SP:(o + 1) * SP], op[:])
            if o == OD // SP - 1:
                nc.sync.dma_start(out_d.ap()[i * P:(i + 1) * P, :], osb[:])

        # ---- attention units (flat QK stream, PV lags cross-unit) ----
        po_tiles = {}
        pts = {}
        norm_state = {}

        def emit_qk(u, jc):
            ih, h = u // NH, u % NH
            dc, hoff = h // 2, (h % 2) * 64
            ps = ps_pool.tile([P, IHALF], F32, tag="ps")
            for sq in range(ISP):
                i0 = ih * IHALF + sq * SP
                nc.tensor.matmul(
                    ps[:, sq * SP:(sq + 1) * SP],
                    kt[hoff:hoff + 64, dc * NK + jc * P: dc * NK + (jc + 1) * P],
                    qt[hoff:hoff + 64, dc * NQ + i0: dc * NQ + i0 + SP],
                    start=True, stop=True)
            pt = pts_pool.tile([P, IHALF], BF16, tag="pt", name=f"pt{u}_{jc}")
            nc.scalar.activation(pt[:], ps[:], AF.Exp, scale=SCALE)
            pts[(u, jc)] = pt

        def emit_pv(u, jc):
            h = u % NH
            if jc == 0:
                po_tiles[u] = po_pool.tile([P, ICH * DH], F32, tag="po",
                                           name=f"po{u}")
            po = po_tiles[u]
            pt = pts[(u, jc)]
            for icl in range(ICH):
                # one accumulation group per PSUM bank: start/stop only on the
                # very first/last matmul touching the region (zero-region rule)
                st = (jc == 0 and icl == 0)
                sp = (jc == JC - 1 and icl == ICH - 1)
                nc.tensor.matmul(
                    po[:, icl * DH:(icl + 1) * DH],
                    pt[:, icl * P:(icl + 1) * P],
                    vaug[:, jc * VW + h * 65: jc * VW + h * 65 + 64],
                    start=st, stop=sp)
                nc.tensor.matmul(
                    pr[:, icl: icl + 1],
                    pt[:, icl * P:(icl + 1) * P],
                    vaug[:, jc * VW + h * 65 + 64: jc * VW + h * 65 + 65],
                    start=st, stop=sp)
            if jc >= 2:
                pts.pop((u, jc - 2), None)

        def emit_norm_a(u):
            """Per-partition reciprocal + normalize to bf16 staging."""
            po = po_tiles[u]
            rv = rv_pool.tile([P, ICH], F32, tag="rv")
            nc.vector.reciprocal(rv[:], pr[:, 0:ICH])
            on = on_pool.tile([P, ICH * DH], BF16, tag="on")
            for icl in range(ICH):
                nc.vector.tensor_scalar_mul(
                    on[:, icl * DH:(icl + 1) * DH],
                    po[:, icl * DH:(icl + 1) * DH], rv[:, icl:icl + 1])
            norm_state[u] = (on, gcur[0])

        def emit_norm_b(u):
            """PE transpose [i,d]->[d,i] and store to ot_all."""
            ih, h = u // NH, u % NH
            dc, hoff = h // 2, (h % 2) * 64
            on, _ = norm_state.pop(u)
            tr = misc_pool.tile([64, ICH * P], BF16, tag="m", name=f"tr{u}")
            for icl in range(ICH):
                nc.tensor.transpose(
                    tr[0:64, icl * P:(icl + 1) * P],
                    on[:, icl * DH:(icl + 1) * DH], ident[:])
            nc.vector.tensor_copy(
                ot_all[hoff:hoff + 64,
                       dc * NQ + ih * IHALF: dc * NQ + (ih + 1) * IHALF],
                tr[0:64, :])
            po_tiles.pop(u)
            if h == NH - 1:
                oproj_avail[0] = max(oproj_avail[0], (ih + 1) * ICH)

        # filler queue: (deadline_unit, emit_fn), pulled inside the jc loop
        # in span-sized pieces so ACT's 2-slot exp buffer never drains.
        fillers = []
        for sq in range(ISP):
            fillers.append((2, qproj_pieces(1, 0, sq)))
        for d in range(2, DC):
            for s0 in range(NSPK):
                fillers.append((2 * d, kproj_pieces(d, s0)))
            for sq in range(ISP):
                fillers.append((2 * d, qproj_pieces(d, 0, sq)))
        if NIH > 1:
            for d in range(DC):
                for sq in range(ISP):
                    fillers.append((NH + 2 * d, qproj_pieces(d, 1, sq)))
        fillers.reverse()  # pop() serves in original order
        microq = []
        ostart = (JC // 2 - 1, JC - 3) if NIH > 1 else ()
        ofin = (JC // 2 + 1, JC - 1) if NIH > 1 else ()

        osb_cur = [None]
        steps = [(u, jc) for u in range(NUNITS) for jc in range(JC)]
        pv_i = [0]
        vp_done = [JC // 4]   # vaug chunks ready (built in kp phase)
        oproj_next = [0]
        oproj_avail = [0]

        def pv_ready(g):
            if pv_i[0] >= len(steps):
                return False
            pu, pj = steps[pv_i[0]]
            if pj >= vp_done[0] and vp_done[0] < JC:
                return False
            return pv_i[0] <= g - 3

        def drain_pv(g):
            while pv_ready(g):
                pu, pj = steps[pv_i[0]]
                emit_pv(pu, pj)
                pv_i[0] += 1
                if pj == JC - 1:
                    emit_norm_a(pu)

        nbh = (min(5, JC - 1), min(11, JC - 1))

        def emit_ready_norm_b(g):
            for v in sorted(norm_state):
                if norm_state[v][1] <= g - 2:
                    emit_norm_b(v)

        gcur = [0]
        for g, (u, jc) in enumerate(steps):
            gcur[0] = g
            emit_qk(u, jc)
            if jc % 2 == 1 and vp_done[0] < JC:
                vproj_into(ps_pool, "ps", vp_done[0])
                vp_done[0] += 1
            elif u > 0:
                if jc in nbh and norm_state:
                    emit_ready_norm_b(g)
                if (jc >= min(4, JC - 2) and jc % 2 == 0 and not microq
                        and fillers and (
                        fillers[-1][0] <= u + 2 or jc <= JC // 2)):
                    microq.extend(fillers.pop()[1])
                if microq and (jc % 2 == 0 or JC <= 8):
                    microq.pop(0)()
                if jc in ostart:
                    if (oproj_next[0] < oproj_avail[0]
                            and osb_cur[0] is None):
                        osb_cur[0] = osb_pool.tile(
                            [P, OD], F32, tag="osb", name=f"osbm{g}")
                        oproj_span(oproj_next[0], 0, osb_cur[0])
                elif jc in ofin and osb_cur[0] is not None:
                    oproj_span(oproj_next[0], OD // SP - 1, osb_cur[0])
                    oproj_next[0] += 1
                    osb_cur[0] = None
            drain_pv(g)

        g = len(steps)
        while pv_i[0] < len(steps):
            gcur[0] = g
            drain_pv(g)
            g += 1
        for v in sorted(norm_state):
            emit_norm_b(v)
        while fillers:
            for piece in fillers.pop()[1]:
                piece()
        while microq:
            microq.pop(0)()
        oproj_avail[0] = IC
        while oproj_next[0] < IC:
            emit_oproj(oproj_next[0], tail=True)
            oproj_next[0] += 1

    nc.compile()
    return nc


def shard_inputs(x, context, mask, Wq, Wk, Wv, Wo):
    """Host-side shard prep: per-core bf16 transposed inputs."""
    bf = ml_dtypes.bfloat16
    in_maps = []
    for c in range(NCORES):
        b, hb = c // 2, c % 2
        cols = slice(hb * HD, (hb + 1) * HD)
        in_maps.append({
            "xt": np.ascontiguousarray(x[b].T).astype(bf),
            "ct": np.ascontiguousarray(context[b].T).astype(bf),
            "wq": np.ascontiguousarray(Wq[:, cols]).astype(bf),
            "wk": np.ascontiguousarray(Wk[:, cols]).astype(bf),
            "wv": np.ascontiguousarray(Wv[:, cols]).astype(bf),
            "wo": np.ascontiguousarray(Wo[cols, :]).astype(bf),
            "mk": mask[b].astype(np.float32),
        })
    return in_maps


_NC_CACHE = {}


def kernel(x, context, mask, Wq, Wk, Wv, Wo, bo, _trace=False):
    x = np.asarray(x, np.float32)
    context = np.asarray(context, np.float32)
    mask = np.asarray(mask)
    Wq, Wk, Wv = (np.asarray(a, np.float32) for a in (Wq, Wk, Wv))
    Wo, bo = np.asarray(Wo, np.float32), np.asarray(bo, np.float32)

    if "nc" not in _NC_CACHE:
        _NC_CACHE["nc"] = build_nc()
    nc = _NC_CACHE["nc"]

    in_maps = shard_inputs(x, context, mask, Wq, Wk, Wv, Wo)
    res = run_bass_kernel_spmd(nc, in_maps, list(range(NCORES)), trace=_trace)
    out = np.zeros((B, N, OD), np.float32)
    for c in range(NCORES):
        out[c // 2] += res.results[c]["out"]
    out += bo
    _NC_CACHE["last_res"] = res
    return out


# revision 28
# speedup vs baseline: 1.2306x; 1.0064x over previous
"""Trainium2 Bass kernel for nn_CrossAttention_7584912245418.

Sharding: batch*head-blocks across 8 cores. Core c handles batch b=c//2 and
head block hb=c%2 (8 of 16 heads). Weights column/row-sliced per head block;
no cross-device communication. Host feeds pre-transposed bf16 activations
(xT, cT); the two per-batch partial outputs (Wo row-split) + bias are summed
on host.

Per-core pipeline (cost-model-aware: matmul cost = out free size x cycle,
so PV is recast as tiny [i,d]-output matmuls; ACT exp ~255us is the other
near-critical engine and the schedule keeps it saturated):
  KT(d0,d1) = Wk_s^T @ cT   k-streamed against the ct DMA (8 PSUM banks)
  V(jc 0..3) projected in the same phase, hiding the xT DMA wait
  QT(d0, ihalf0) k-streamed against xT; rest of Q/K/V proj becomes filler
  units = (i-half, head), flat QK stream with cross-unit PV lag 3:
    per jc: S^T[j,i] = KT_chunk^T @ QT     (PSUM f32 [128, 1024])
            P^T = exp(SCALE*S^T)           (ACT, bf16, no max-sub)
            O[i,d] += P^T_chunk^T @ V      (65-row matmuls; rowsum via
            rowsum[i] += P^T_chunk^T @ mask col     one extra 1-row matmul)
    leftover K/Q projections ride as 4-matmul micro-pieces on even steps
    (never outrunning ACT's 2-exp pipeline buffer); V proj for jc 4..15
    rides odd steps of units 0-1.
    norm: per-partition recip+mul (no gpsimd broadcast), PE transpose
          (identity matmul) back to [d,i] bf16 ot_all.
  out = OT^T @ Wo per i-chunk; i-half 0's output projection hides inside
  i-half 1's exp window; the last unit runs as two 512-query sub-units so
  half its output projection hides inside the final exp window.
PSUM: ps 2x2 banks (QK/exp double buffer), po 2x1 (one accumulation group
per bank per unit), pr 1 (rowsums), misc 1 (fillers/oproj/transposes).
"""

import sys

for _p in ("/opt/trn_rl_repo",):
    if _p not in sys.path:
        sys.path.insert(0, _p)

from contextlib import ExitStack

import ml_dtypes
import numpy as np

import concourse.bass as bass
import concourse.mybir as mybir
import concourse.tile as tile
from concourse import bacc
from concourse.bass_utils import run_bass_kernel_spmd
from concourse.masks import make_identity

F32 = mybir.dt.float32
BF16 = mybir.dt.bfloat16
AF = mybir.ActivationFunctionType

# Full-problem constants
B, N, M = 4, 2048, 2048
QD, CD, OD = 1024, 1024, 1024
H, DH = 16, 64
SCALE = DH ** -0.5
NCORES = 8
NH = 8            # heads per core
HD = NH * DH      # 512, per-core inner dim
P = 128


def build_nc(NQ=N, NK=M, KD=QD, trace_sim=False):
    """Build the per-core SPMD program. NQ=query len, NK=kv len, KD=model dim."""
    KC = KD // P            # contraction chunks for projections
    JC = NK // P            # key-position chunks
    IC = NQ // P            # query-position chunks
    SP = 512                # projection matmul free-dim span
    IHALF = min(1024, NQ)   # queries per attention unit (and exp granularity)
    NIH = NQ // IHALF
    ISP = IHALF // SP       # spans per i-half
    ICH = IHALF // P        # i-chunks per half
    DC = HD // P            # 4 head-pair chunks (2 heads per chunk)
    VW = NH * 65            # vaug row width per j-chunk
    NUNITS = NIH * NH

    nc = bacc.Bacc("TRN2", target_bir_lowering=False, debug=False,
                   enable_asserts=False)

    xt_d = nc.dram_tensor("xt", [KD, NQ], BF16, kind="ExternalInput")
    ct_d = nc.dram_tensor("ct", [KD, NK], BF16, kind="ExternalInput")
    wq_d = nc.dram_tensor("wq", [KD, HD], BF16, kind="ExternalInput")
    wk_d = nc.dram_tensor("wk", [KD, HD], BF16, kind="ExternalInput")
    wv_d = nc.dram_tensor("wv", [KD, HD], BF16, kind="ExternalInput")
    wo_d = nc.dram_tensor("wo", [HD, OD], BF16, kind="ExternalInput")
    mk_d = nc.dram_tensor("mk", [NK], F32, kind="ExternalInput")
    out_d = nc.dram_tensor("out", [NQ, OD], F32, kind="ExternalOutput")

    with tile.TileContext(nc, trace_sim=trace_sim) as tc, ExitStack() as ctx:
        # ---- persistent SBUF ----
        pp = ctx.enter_context(tc.tile_pool(name="persist", bufs=1))
        kt = pp.tile([P, DC * NK], BF16, tag="kt")
        qt = pp.tile([P, DC * NQ], BF16, tag="qt")
        vaug = pp.tile([P, JC * VW], BF16, tag="vaug")
        mkt = pp.tile([P, JC], F32, tag="mkt")
        ot_all = pp.tile([P, DC * NQ], BF16, tag="ot_all")
        ident = pp.tile([P, P], BF16, tag="ident")
        wo_t = pp.tile([P, DC * OD], BF16, tag="wo_t")
        cts = [pp.tile([P, NK], BF16, tag=f"ct{k}", name=f"ct{k}")
               for k in range(KC)]
        wks = [pp.tile([P, HD], BF16, tag=f"wk{k}", name=f"wk{k}")
               for k in range(KC)]
        wvs = [pp.tile([P, HD], BF16, tag=f"wv{k}", name=f"wv{k}")
               for k in range(KC)]
        xts = [pp.tile([P, NQ], BF16, tag=f"xt{k}", name=f"xt{k}")
               for k in range(KC)]
        wqs = [pp.tile([P, HD], BF16, tag=f"wq{k}", name=f"wq{k}")
               for k in range(KC)]

        pts_pool = ctx.enter_context(tc.tile_pool(name="pts", bufs=12))
        on_pool = ctx.enter_context(tc.tile_pool(name="on", bufs=2))
        rv_pool = ctx.enter_context(tc.tile_pool(name="rv", bufs=2))
        osb_pool = ctx.enter_context(tc.tile_pool(name="osb", bufs=3))

        # ---- input DMAs (order = DMA device order: K-proj inputs first) ----
        for k in range(KC):
            r = slice(k * P, (k + 1) * P)
            nc.sync.dma_start(wks[k][:], wk_d.ap()[r, :])
            nc.sync.dma_start(cts[k][:], ct_d.ap()[r, :])
            nc.sync.dma_start(wvs[k][:], wv_d.ap()[r, :])
        nc.sync.dma_start(mkt[:], mk_d.ap().rearrange("(jc p) -> p jc", p=P))
        for k in range(KC):
            r = slice(k * P, (k + 1) * P)
            nc.sync.dma_start(wqs[k][:, 0:P], wq_d.ap()[r, 0:P])
            nc.sync.dma_start(xts[k][:, 0:IHALF], xt_d.ap()[r, 0:IHALF])
        for k in range(KC):
            r = slice(k * P, (k + 1) * P)
            nc.sync.dma_start(wqs[k][:, P:HD], wq_d.ap()[r, P:HD])
            if NQ > IHALF:
                nc.sync.dma_start(xts[k][:, IHALF:NQ], xt_d.ap()[r, IHALF:NQ])
        for d in range(DC):
            nc.sync.dma_start(wo_t[:, d * OD:(d + 1) * OD],
                              wo_d.ap()[d * P:(d + 1) * P, :])

        make_identity(nc, ident[:])
        warm = pp.tile([1, 1], BF16, tag="warm")
        nc.scalar.activation(warm[:], ident[0:1, 0:1], AF.Exp, scale=1.0)
        vz = vaug[:].rearrange("p (jc h e) -> p (jc h) e", jc=JC, h=NH)
        nc.vector.memset(vz[:, :, 64:65], 1.0)

        # ---- K-proj d=0,1 k-streamed against the ct DMA; then Q(d0, ih0) ----
        NSPK = NK // SP
        with tc.tile_pool(name="kp", bufs=2 * NSPK, space="PSUM") as kp:
            ka = [kp.tile([P, SP], F32, tag="ka", name=f"ka{t}")
                  for t in range(2 * NSPK)]
            for k in range(KC):
                for d in range(2):
                    for s in range(NSPK):
                        nc.tensor.matmul(
                            ka[d * NSPK + s][:], wks[k][:, d * P:(d + 1) * P],
                            cts[k][:, s * SP:(s + 1) * SP],
                            start=(k == 0), stop=(k == KC - 1))
            for d in range(2):
                for s in range(NSPK):
                    nc.vector.tensor_copy(
                        kt[:, d * NK + s * SP: d * NK + (s + 1) * SP],
                        ka[d * NSPK + s][:])
            for jc in range(JC // 4):
                vp = kp.tile([P, HD], F32, tag="ka", name=f"vpk{jc}")
                for k in range(KC):
                    nc.tensor.matmul(
                        vp[:], cts[k][:, jc * P:(jc + 1) * P], wvs[k][:],
                        start=(k == 0), stop=(k == KC - 1))
                dstv = vaug[:, jc * VW:(jc + 1) * VW].rearrange(
                    "p (h e) -> p h e", h=NH)
                nc.vector.tensor_scalar_mul(
                    dstv[:, :, 0:64],
                    vp[:].rearrange("p (h e) -> p h e", h=NH),
                    mkt[:, jc:jc + 1])
                nc.vector.tensor_scalar_mul(
                    dstv[:, :, 64:65], dstv[:, :, 64:65], mkt[:, jc:jc + 1])
            for s in range(ISP):
                q0 = kp.tile([P, SP], F32, tag="ka", name=f"q0_{s}")
                for k in range(KC):
                    nc.tensor.matmul(
                        q0[:], wqs[k][:, 0:P], xts[k][:, s * SP:(s + 1) * SP],
                        start=(k == 0), stop=(k == KC - 1))
                nc.vector.tensor_copy(qt[:, s * SP:(s + 1) * SP], q0[:])

        # ---- attention-phase PSUM pools ----
        ps_pool = ctx.enter_context(
            tc.tile_pool(name="ps", bufs=2, space="PSUM"))
        po_pool = ctx.enter_context(
            tc.tile_pool(name="po", bufs=2, space="PSUM"))
        prp = ctx.enter_context(tc.tile_pool(name="pr", bufs=1, space="PSUM"))
        pr = prp.tile([P, ICH], F32, tag="pr")
        misc_pool = ctx.enter_context(
            tc.tile_pool(name="misc", bufs=1, space="PSUM"))

        # ---- filler emitters (PE work to hide under the exp pipeline) ----
        # Span tasks split into 2 pieces of KC/2 matmuls each so a filler
        # burst never outruns ACT's short exp pipeline buffer.
        def _pieces(alloc, mm, fin):
            st = {}

            def p1():
                st["t"] = alloc()
                mm(st["t"], 0, KC // 2)

            def p2():
                mm(st["t"], KC // 2, KC)
                fin(st["t"])
            return [p1, p2]

        def kproj_pieces(d, s0):
            def alloc():
                return misc_pool.tile([P, SP], F32, tag="m", name=f"kp{d}_{s0}")

            def mm(t, k0, k1):
                for k in range(k0, k1):
                    nc.tensor.matmul(
                        t[:], wks[k][:, d * P:(d + 1) * P],
                        cts[k][:, s0 * SP:(s0 + 1) * SP],
                        start=(k == 0), stop=(k == KC - 1))

            def fin(t):
                nc.vector.tensor_copy(
                    kt[:, d * NK + s0 * SP: d * NK + (s0 + 1) * SP], t[:])
            return _pieces(alloc, mm, fin)

        def qproj_pieces(d, ih, sq):
            c0 = ih * IHALF + sq * SP

            def alloc():
                return misc_pool.tile([P, SP], F32, tag="m",
                                      name=f"qp{d}_{ih}_{sq}")

            def mm(t, k0, k1):
                for k in range(k0, k1):
                    nc.tensor.matmul(
                        t[:], wqs[k][:, d * P:(d + 1) * P],
                        xts[k][:, c0:c0 + SP],
                        start=(k == 0), stop=(k == KC - 1))

            def fin(t):
                nc.vector.tensor_copy(
                    qt[:, d * NQ + c0: d * NQ + c0 + SP], t[:])
            return _pieces(alloc, mm, fin)

        def vproj_into(pool, tag, jc):
            """V projection for one j-chunk into vaug via a [P, HD] psum tile."""
            ps = pool.tile([P, HD], F32, tag=tag, name=f"vp{jc}")
            for k in range(KC):
                nc.tensor.matmul(
                    ps[:], cts[k][:, jc * P:(jc + 1) * P], wvs[k][:],
                    start=(k == 0), stop=(k == KC - 1))
            dst = vaug[:, jc * VW:(jc + 1) * VW].rearrange(
                "p (h e) -> p h e", h=NH)
            nc.vector.tensor_scalar_mul(
                dst[:, :, 0:64],
                ps[:].rearrange("p (h e) -> p h e", h=NH),
                mkt[:, jc:jc + 1])
            nc.vector.tensor_scalar_mul(
                dst[:, :, 64:65], dst[:, :, 64:65], mkt[:, jc:jc + 1])

        def emit_oproj(i, tail=False):
            """Output projection for global i-chunk i."""
            if tail:
                op = ps_pool.tile([P, OD], F32, tag="ps", name=f"op{i}")
                for o in range(OD // SP):
                    for d in range(DC):
                        nc.tensor.matmul(
                            op[:, o * SP:(o + 1) * SP],
                            ot_all[:, d * NQ + i * P: d * NQ + (i + 1) * P],
                            wo_t[:, d * OD + o * SP: d * OD + (o + 1) * SP],
                            start=(d == 0), stop=(d == DC - 1))
                osb = osb_pool.tile([P, OD], F32, tag="osb")
                nc.vector.tensor_copy(osb[:], op[:])
                nc.sync.dma_start(out_d.ap()[i * P:(i + 1) * P, :], osb[:])
                return
            osb = osb_pool.tile([P, OD], F32, tag="osb")
            if True:
                for o in range(OD // SP):
                    op = misc_pool.tile([P, SP], F32, tag="m",
                                        name=f"op{i}_{o}")
                    for d in range(DC):
                        nc.tensor.matmul(
                            op[:],
                            ot_all[:, d * NQ + i * P: d * NQ + (i + 1) * P],
                            wo_t[:, d * OD + o * SP: d * OD + (o + 1) * SP],
                            start=(d == 0), stop=(d == DC - 1))
                    nc.vector.tensor_copy(osb[:, o * SP:(o + 1) * SP], op[:])
            nc.sync.dma_start(out_d.ap()[i * P:(i + 1) * P, :], osb[:])

        def oproj_span(i, o, osb):
            op = misc_pool.tile([P, SP], F32, tag="m", name=f"op{i}_{o}")
            for d in range(DC):
                nc.tensor.matmul(
                    op[:], ot_all[:, d * NQ + i * P: d * NQ + (i + 1) * P],
                    wo_t[:, d * OD + o * SP: d * OD + (o + 1) * SP],
                    start=(d == 0), stop=(d == DC - 1))
            nc.vector.tensor_copy(osb[:, o * SP:(o + 1) * SP], op[:])
            if o == OD // SP - 1:
                nc.sync.dma_start(out_d.ap()[i * P:(i + 1) * P, :], osb[:])

        # ---- attention units (flat QK stream, PV lags cross-unit) ----
        po_tiles = {}
        pts = {}
        norm_state = {}

        def emit_qk(u, jc):
            ih, h = u // NH, u % NH
            dc, hoff = h // 2, (h % 2) * 64
            ps = ps_pool.tile([P, IHALF], F32, tag="ps")
            for sq in range(ISP):
                i0 = ih * IHALF + sq * SP
                nc.tensor.matmul(
                    ps[:, sq * SP:(sq + 1) * SP],
                    kt[hoff:hoff + 64, dc * NK + jc * P: dc * NK + (jc + 1) * P],
                    qt[hoff:hoff + 64, dc * NQ + i0: dc * NQ + i0 + SP],
                    start=True, stop=True)
            pt = pts_pool.tile([P, IHALF], BF16, tag="pt", name=f"pt{u}_{jc}")
            nc.scalar.activation(pt[:], ps[:], AF.Exp, scale=SCALE)
            pts[(u, jc)] = pt

        def emit_pv(u, jc):
            h = u % NH
            if jc == 0:
                po_tiles[u] = po_pool.tile([P, ICH * DH], F32, tag="po",
                                           name=f"po{u}")
            po = po_tiles[u]
            pt = pts[(u, jc)]
            for icl in range(ICH):
                # one accumulation group per PSUM bank: start/stop only on the
                # very first/last matmul touching the region (zero-region rule)
                st = (jc == 0 and icl == 0)
                sp = (jc == JC - 1 and icl == ICH - 1)
                nc.tensor.matmul(
                    po[:, icl * DH:(icl + 1) * DH],
                    pt[:, icl * P:(icl + 1) * P],
                    vaug[:, jc * VW + h * 65: jc * VW + h * 65 + 64],
                    start=st, stop=sp)
                nc.tensor.matmul(
                    pr[:, icl: icl + 1],
                    pt[:, icl * P:(icl + 1) * P],
                    vaug[:, jc * VW + h * 65 + 64: jc * VW + h * 65 + 65],
                    start=st, stop=sp)
            if jc >= 2:
                pts.pop((u, jc - 2), None)

        def emit_norm_a(u):
            """Per-partition reciprocal + normalize to bf16 staging."""
            po = po_tiles[u]
            rv = rv_pool.tile([P, ICH], F32, tag="rv")
            nc.vector.reciprocal(rv[:], pr[:, 0:ICH])
            on = on_pool.tile([P, ICH * DH], BF16, tag="on")
            for icl in range(ICH):
                nc.vector.tensor_scalar_mul(
                    on[:, icl * DH:(icl + 1) * DH],
                    po[:, icl * DH:(icl + 1) * DH], rv[:, icl:icl + 1])
            norm_state[u] = (on, gcur[0])

        def emit_norm_b(u):
            """PE transpose [i,d]->[d,i] and store to ot_all."""
            ih, h = u // NH, u % NH
            dc, hoff = h // 2, (h % 2) * 64
            on, _ = norm_state.pop(u)
            tr = misc_pool.tile([64, ICH * P], BF16, tag="m", name=f"tr{u}")
            for icl in range(ICH):
                nc.tensor.transpose(
                    tr[0:64, icl * P:(icl + 1) * P],
                    on[:, icl * DH:(icl + 1) * DH], ident[:])
            nc.vector.tensor_copy(
                ot_all[hoff:hoff + 64,
                       dc * NQ + ih * IHALF: dc * NQ + (ih + 1) * IHALF],
                tr[0:64, :])
            po_tiles.pop(u)
            if h == NH - 1:
                oproj_avail[0] = max(oproj_avail[0], (ih + 1) * ICH)

        # filler queue: (deadline_unit, emit_fn), pulled inside the jc loop
        # in span-sized pieces so ACT's 2-slot exp buffer never drains.
        fillers = []
        for sq in range(ISP):
            fillers.append((2, qproj_pieces(1, 0, sq)))
        for d in range(2, DC):
            for s0 in range(NSPK):
                fillers.append((2 * d, kproj_pieces(d, s0)))
            for sq in range(ISP):
                fillers.append((2 * d, qproj_pieces(d, 0, sq)))
        if NIH > 1:
            for d in range(DC):
                for sq in range(ISP):
                    fillers.append((NH + 2 * d, qproj_pieces(d, 1, sq)))
        fillers.reverse()  # pop() serves in original order
        microq = []
        ostart = (JC // 2 - 1, JC - 3) if NIH > 1 else ()
        ofin = (JC // 2 + 1, JC - 1) if NIH > 1 else ()

        osb_cur = [None]
        last_special = NIH > 1 and ICH % 2 == 0
        main_units = NUNITS - 1 if last_special else NUNITS
        steps = [(u, jc) for u in range(main_units) for jc in range(JC)]
        pv_i = [0]
        vp_done = [JC // 4]   # vaug chunks ready (built in kp phase)
        oproj_next = [0]
        oproj_avail = [0]

        def pv_ready(g):
            if pv_i[0] >= len(steps):
                return False
            pu, pj = steps[pv_i[0]]
            if pj >= vp_done[0] and vp_done[0] < JC:
                return False
            return pv_i[0] <= g - 3

        def drain_pv(g):
            while pv_ready(g):
                pu, pj = steps[pv_i[0]]
                emit_pv(pu, pj)
                pv_i[0] += 1
                if pj == JC - 1:
                    emit_norm_a(pu)

        nbh = (min(5, JC - 1), min(11, JC - 1))

        def emit_ready_norm_b(g):
            for v in sorted(norm_state):
                if norm_state[v][1] <= g - 2:
                    emit_norm_b(v)

        gcur = [0]
        for g, (u, jc) in enumerate(steps):
            gcur[0] = g
            emit_qk(u, jc)
            if jc % 2 == 1 and vp_done[0] < JC:
                vproj_into(ps_pool, "ps", vp_done[0])
                vp_done[0] += 1
            elif u > 0:
                if jc in nbh and norm_state:
                    emit_ready_norm_b(g)
                if (jc >= min(4, JC - 2) and jc % 2 == 0 and not microq
                        and fillers and (
                        fillers[-1][0] <= u + 2 or jc <= JC // 2)):
                    microq.extend(fillers.pop()[1])
                if microq and (jc % 2 == 0 or JC <= 8):
                    microq.pop(0)()
                if jc in ostart:
                    if (oproj_next[0] < oproj_avail[0]
                            and osb_cur[0] is None):
                        osb_cur[0] = osb_pool.tile(
                            [P, OD], F32, tag="osb", name=f"osbm{g}")
                        oproj_span(oproj_next[0], 0, osb_cur[0])
                elif jc in ofin and osb_cur[0] is not None:
                    oproj_span(oproj_next[0], OD // SP - 1, osb_cur[0])
                    oproj_next[0] += 1
                    osb_cur[0] = None
            drain_pv(g)

        if last_special:
            # Last unit runs as two 512-query sub-units so its first half's
            # output projection hides inside the second half's exp window.
            HQ = IHALF // 2
            HICH = ICH // 2
            L = NUNITS - 1
            lih, lh = L // NH, L % NH
            ldc, lhoff = lh // 2, (lh % 2) * 64
            sub_pts = {}
            sub_po = {}
            sub_on = {}
            sub_pvi = [0]
            sub_steps = [(iq, jc) for iq in range(2) for jc in range(JC)]

            def sub_qk(iq, jc):
                ps = ps_pool.tile([P, HQ], F32, tag="ps")
                i0 = lih * IHALF + iq * HQ
                nc.tensor.matmul(
                    ps[:],
                    kt[lhoff:lhoff + 64,
                       ldc * NK + jc * P: ldc * NK + (jc + 1) * P],
                    qt[lhoff:lhoff + 64, ldc * NQ + i0: ldc * NQ + i0 + HQ],
                    start=True, stop=True)
                pt = pts_pool.tile([P, HQ], BF16, tag="pt",
                                   name=f"spt{iq}_{jc}")
                nc.scalar.activation(pt[:], ps[:], AF.Exp, scale=SCALE)
                sub_pts[(iq, jc)] = pt

            def sub_pv(iq, jc):
                if jc == 0:
                    sub_po[iq] = po_pool.tile([P, HICH * DH], F32, tag="po",
                                              name=f"spo{iq}")
                po = sub_po[iq]
                pt = sub_pts[(iq, jc)]
                for icl in range(HICH):
                    st = (jc == 0 and icl == 0)
                    sp = (jc == JC - 1 and icl == HICH - 1)
                    nc.tensor.matmul(
                        po[:, icl * DH:(icl + 1) * DH],
                        pt[:, icl * P:(icl + 1) * P],
                        vaug[:, jc * VW + lh * 65: jc * VW + lh * 65 + 64],
                        start=st, stop=sp)
                    nc.tensor.matmul(
                        pr[:, iq * HICH + icl: iq * HICH + icl + 1],
                        pt[:, icl * P:(icl + 1) * P],
                        vaug[:, jc * VW + lh * 65 + 64:
                             jc * VW + lh * 65 + 65],
                        start=st, stop=sp)

            def sub_norm_a(iq):
                po = sub_po[iq]
                rv = rv_pool.tile([P, HICH], F32, tag="rv")
                nc.vector.reciprocal(rv[:], pr[:, iq * HICH:(iq + 1) * HICH])
                on = on_pool.tile([P, HICH * DH], BF16, tag="on")
                for icl in range(HICH):
                    nc.vector.tensor_scalar_mul(
                        on[:, icl * DH:(icl + 1) * DH],
                        po[:, icl * DH:(icl + 1) * DH], rv[:, icl:icl + 1])
                sub_on[iq] = on

            def sub_norm_b(iq):
                on = sub_on.pop(iq)
                tr = misc_pool.tile([64, HICH * P], BF16, tag="m",
                                    name=f"str{iq}")
                for icl in range(HICH):
                    nc.tensor.transpose(
                        tr[0:64, icl * P:(icl + 1) * P],
                        on[:, icl * DH:(icl + 1) * DH], ident[:])
                i0 = lih * IHALF + iq * HQ
                nc.vector.tensor_copy(
                    ot_all[lhoff:lhoff + 64, ldc * NQ + i0: ldc * NQ + i0 + HQ],
                    tr[0:64, :])
                oproj_avail[0] = max(oproj_avail[0], (i0 + HQ) // P)

            def sub_drain(gg):
                while sub_pvi[0] < len(sub_steps) and sub_pvi[0] <= gg - 2:
                    siq, sjc = sub_steps[sub_pvi[0]]
                    sub_pv(siq, sjc)
                    sub_pvi[0] += 1
                    if sjc == JC - 1:
                        sub_norm_a(siq)

            for gg, (iq, jc) in enumerate(sub_steps):
                sub_qk(iq, jc)
                gcur[0] = len(steps) + gg
                drain_pv(len(steps) + gg)
                if iq == 0 and jc == 4 and norm_state:
                    emit_ready_norm_b(len(steps) + gg)
                if iq == 1:
                    if jc in (2, 3) and 0 in sub_on:
                        sub_norm_b(0)
                    if jc in (3, 7, 11):
                        if (oproj_next[0] < oproj_avail[0]
                                and osb_cur[0] is None):
                            osb_cur[0] = osb_pool.tile(
                                [P, OD], F32, tag="osb", name=f"osbs{gg}")
                            oproj_span(oproj_next[0], 0, osb_cur[0])
                    elif jc in (5, 9, 13) and osb_cur[0] is not None:
                        oproj_span(oproj_next[0], OD // SP - 1, osb_cur[0])
                        oproj_next[0] += 1
                        osb_cur[0] = None
                sub_drain(gg)
            gg = len(sub_steps)
            while sub_pvi[0] < len(sub_steps):
                sub_drain(gg)
                gg += 1
            for v in sorted(norm_state):
                emit_norm_b(v)
            if 1 in sub_on:
                sub_norm_b(1)
        else:
            g = len(steps)
            while pv_i[0] < len(steps):
                gcur[0] = g
                drain_pv(g)
                g += 1
            for v in sorted(norm_state):
                emit_norm_b(v)
        while fillers:
            for piece in fillers.pop()[1]:
                piece()
        while microq:
            microq.pop(0)()
        oproj_avail[0] = IC
        while oproj_next[0] < IC:
            emit_oproj(oproj_next[0], tail=True)
            oproj_next[0] += 1

    nc.compile()
    return nc


def shard_inputs(x, context, mask, Wq, Wk, Wv, Wo):
    """Host-side shard prep: per-core bf16 transposed inputs."""
    bf = ml_dtypes.bfloat16
    in_maps = []
    for c in range(NCORES):
        b, hb = c // 2, c % 2
        cols = slice(hb * HD, (hb + 1) * HD)
        in_maps.append({
            "xt": np.ascontiguousarray(x[b].T).astype(bf),
            "ct": np.ascontiguousarray(context[b].T).astype(bf),
            "wq": np.ascontiguousarray(Wq[:, cols]).astype(bf),
            "wk": np.ascontiguousarray(Wk[:, cols]).astype(bf),
            "wv": np.ascontiguousarray(Wv[:, cols]).astype(bf),
            "wo": np.ascontiguousarray(Wo[cols, :]).astype(bf),
            "mk": mask[b].astype(np.float32),
        })
    return in_maps


_NC_CACHE = {}


def kernel(x, context, mask, Wq, Wk, Wv, Wo, bo, _trace=False):
    x = np.asarray(x, np.float32)
    context = np.asarray(context, np.float32)
    mask = np.asarray(mask)
    Wq, Wk, Wv = (np.asarray(a, np.float32) for a in (Wq, Wk, Wv))
    Wo, bo = np.asarray(Wo, np.float32), np.asarray(bo, np.float32)

    if "nc" not in _NC_CACHE:
        _NC_CACHE["nc"] = build_nc()
    nc = _NC_CACHE["nc"]

    in_maps = shard_inputs(x, context, mask, Wq, Wk, Wv, Wo)
    res = run_bass_kernel_spmd(nc, in_maps, list(range(NCORES)), trace=_trace)
    out = np.zeros((B, N, OD), np.float32)
    for c in range(NCORES):
        out[c // 2] += res.results[c]["out"]
    out += bo
    _NC_CACHE["last_res"] = res
    return out


# revision 31
# speedup vs baseline: 1.2629x; 1.0262x over previous
"""Trainium2 Bass kernel for nn_CrossAttention_7584912245418.

Sharding: batch*head-blocks across 8 cores. Core c handles batch b=c//2 and
head block hb=c%2 (8 of 16 heads). Weights column/row-sliced per head block;
no cross-device communication. Host feeds pre-transposed bf16 activations
(xT, cT); the two per-batch partial outputs (Wo row-split) + bias are summed
on host.

Per-core pipeline (cost-model-aware: matmul cost = out free size x cycle,
so PV is recast as tiny [i,d]-output matmuls; ACT exp ~255us is the other
near-critical engine and the schedule keeps it saturated):
  KT(d0,d1) = Wk_s^T @ cT   k-streamed against the ct DMA (8 PSUM banks)
  V(jc 0..3) projected in the same phase, hiding the xT DMA wait
  QT(d0, ihalf0) k-streamed against xT; rest of Q/K/V proj becomes filler
  units = (i-half, head), flat QK stream with cross-unit PV lag 3:
    per jc: S^T[j,i] = KT_chunk^T @ QT     (PSUM f32 [128, 1024])
            P^T = exp(SCALE*S^T)           (ACT, bf16, no max-sub)
            O[i,d] += P^T_chunk^T @ V      (65-row matmuls; rowsum via
            rowsum[i] += P^T_chunk^T @ mask col     one extra 1-row matmul)
    leftover K/Q projections ride as 4-matmul micro-pieces on even steps
    (never outrunning ACT's 2-exp pipeline buffer); V proj for jc 4..15
    rides odd steps of units 0-1.
    norm: per-partition recip+mul (no gpsimd broadcast), PE transpose
          (identity matmul) back to [d,i] bf16 ot_all.
  out = OT^T @ Wo per i-chunk; i-half 0's output projection hides inside
  i-half 1's exp window; the last unit runs as two 512-query sub-units so
  half its output projection hides inside the final exp window.
PSUM: ps 2x2 banks (QK/exp double buffer), po 2x1 (one accumulation group
per bank per unit), pr 1 (rowsums), misc 1 (fillers/oproj/transposes).
"""

import sys

for _p in ("/opt/trn_rl_repo",):
    if _p not in sys.path:
        sys.path.insert(0, _p)

from contextlib import ExitStack

import ml_dtypes
import numpy as np

import concourse.bass as bass
import concourse.mybir as mybir
import concourse.tile as tile
from concourse import bacc
from concourse.bass_utils import run_bass_kernel_spmd
from concourse.masks import make_identity

F32 = mybir.dt.float32
BF16 = mybir.dt.bfloat16
F8 = mybir.dt.float8e4
WSCALE = 64.0           # host scales W{q,k,v} by 64 into fp8 range
AF = mybir.ActivationFunctionType

# Full-problem constants
B, N, M = 4, 2048, 2048
QD, CD, OD = 1024, 1024, 1024
H, DH = 16, 64
SCALE = DH ** -0.5
NCORES = 8
NH = 8            # heads per core
HD = NH * DH      # 512, per-core inner dim
P = 128


def build_nc(NQ=N, NK=M, KD=QD, trace_sim=False):
    """Build the per-core SPMD program. NQ=query len, NK=kv len, KD=model dim."""
    KC = KD // P            # contraction chunks for projections
    JC = NK // P            # key-position chunks
    IC = NQ // P            # query-position chunks
    SP = 512                # projection matmul free-dim span
    IHALF = min(1024, NQ)   # queries per attention unit (and exp granularity)
    NIH = NQ // IHALF
    ISP = IHALF // SP       # spans per i-half
    ICH = IHALF // P        # i-chunks per half
    DC = HD // P            # 4 head-pair chunks (2 heads per chunk)
    VW = NH * 65            # vaug row width per j-chunk
    NUNITS = NIH * NH

    nc = bacc.Bacc("TRN2", target_bir_lowering=False, debug=False,
                   enable_asserts=False)

    xt_ds = [nc.dram_tensor(n, [KD, NQ], F8, kind="ExternalInput")
             for n in ("xth", "xtl")]
    ct_ds = [nc.dram_tensor(n, [KD, NK], F8, kind="ExternalInput")
             for n in ("cth", "ctl")]
    wq_ds = [nc.dram_tensor(n, [KD, HD], F8, kind="ExternalInput")
             for n in ("wqh", "wql")]
    wk_ds = [nc.dram_tensor(n, [KD, HD], F8, kind="ExternalInput")
             for n in ("wkh", "wkl")]
    wv_ds = [nc.dram_tensor(n, [KD, HD], F8, kind="ExternalInput")
             for n in ("wvh", "wvl")]
    wo_d = nc.dram_tensor("wo", [HD, OD], BF16, kind="ExternalInput")
    mk_d = nc.dram_tensor("mk", [NK], F32, kind="ExternalInput")
    out_d = nc.dram_tensor("out", [NQ, OD], F32, kind="ExternalOutput")
    KCD = KD // 256         # DoubleRow contraction chunks (256 rows each)
    DR = mybir.MatmulPerfMode.DoubleRow

    with tile.TileContext(nc, trace_sim=trace_sim) as tc, ExitStack() as ctx:
        # ---- persistent SBUF ----
        pp = ctx.enter_context(tc.tile_pool(name="persist", bufs=1))
        kt = pp.tile([P, DC * NK], BF16, tag="kt")
        qt = pp.tile([P, DC * NQ], BF16, tag="qt")
        vaug = pp.tile([P, JC * VW], BF16, tag="vaug")
        mkt = pp.tile([P, JC], F32, tag="mkt")
        ot_all = pp.tile([P, DC * NQ], BF16, tag="ot_all")
        ident = pp.tile([P, P], BF16, tag="ident")
        wo_t = pp.tile([P, DC * OD], BF16, tag="wo_t")
        def dr_tiles(nm, cols):
            # [hi/lo][chunk] tiles laid out [P, 2, cols] for DoubleRow
            return [[pp.tile([P, 2 * cols], F8, tag=f"{nm}{hl}{c}",
                             name=f"{nm}{hl}{c}") for c in range(KCD)]
                    for hl in range(2)]

        cts = dr_tiles("ct", NK)
        wks = dr_tiles("wk", HD)
        wvs = dr_tiles("wv", HD)
        xts = dr_tiles("xt", NQ)
        wqs = dr_tiles("wq", HD)

        def dr3(t, cols):
            return t[:].rearrange("p (t n) -> p t n", t=2)

        def dr_dma(tile, dram, c, c0, c1):
            nc.sync.dma_start(
                dr3(tile, 0)[:, :, c0:c1],
                dram.ap()[256 * c: 256 * (c + 1), c0:c1].rearrange(
                    "(t p) n -> p t n", t=2))

        pts_pool = ctx.enter_context(tc.tile_pool(name="pts", bufs=12))
        on_pool = ctx.enter_context(tc.tile_pool(name="on", bufs=2))
        rv_pool = ctx.enter_context(tc.tile_pool(name="rv", bufs=2))
        osb_pool = ctx.enter_context(tc.tile_pool(name="osb", bufs=3))

        # ---- input DMAs (order = DMA device order: K-proj inputs first) ----
        for c in range(KCD):
            for hl in range(2):
                dr_dma(wks[hl][c], wk_ds[hl], c, 0, HD)
                dr_dma(cts[hl][c], ct_ds[hl], c, 0, NK)
                dr_dma(wvs[hl][c], wv_ds[hl], c, 0, HD)
        nc.sync.dma_start(mkt[:], mk_d.ap().rearrange("(jc p) -> p jc", p=P))
        for c in range(KCD):
            for hl in range(2):
                dr_dma(wqs[hl][c], wq_ds[hl], c, 0, P)
                dr_dma(xts[hl][c], xt_ds[hl], c, 0, IHALF)
        for c in range(KCD):
            for hl in range(2):
                dr_dma(wqs[hl][c], wq_ds[hl], c, P, HD)
                if NQ > IHALF:
                    dr_dma(xts[hl][c], xt_ds[hl], c, IHALF, NQ)
        for d in range(DC):
            nc.sync.dma_start(wo_t[:, d * OD:(d + 1) * OD],
                              wo_d.ap()[d * P:(d + 1) * P, :])

        make_identity(nc, ident[:])
        warm = pp.tile([1, 1], BF16, tag="warm")
        nc.scalar.activation(warm[:], ident[0:1, 0:1], AF.Exp, scale=1.0)
        vz = vaug[:].rearrange("p (jc h e) -> p (jc h) e", jc=JC, h=NH)
        nc.vector.memset(vz[:, :, 64:65], WSCALE)

        # Each projection matmul = 3 compensated fp8 DoubleRow terms:
        # hi*hi + hi*lo + lo*hi (lo*lo dropped, ~1e-6 relative).
        TERMS = ((0, 0), (0, 1), (1, 0))

        def dr_mm(out, wt, xt_, c, wsl, xsl, start, stop):
            for ti, (wh, xh) in enumerate(TERMS):
                nc.tensor.matmul(
                    out, dr3(wt[wh][c], 0)[:, :, wsl[0]:wsl[1]],
                    dr3(xt_[xh][c], 0)[:, :, xsl[0]:xsl[1]],
                    start=(start and ti == 0), stop=(stop and ti == 2),
                    perf_mode=DR)

        # ---- K-proj d=0,1 k-streamed against the ct DMA; then Q(d0, ih0) ----
        NSPK = NK // SP
        with tc.tile_pool(name="kp", bufs=2 * NSPK, space="PSUM") as kp:
            ka = [kp.tile([P, SP], F32, tag="ka", name=f"ka{t}")
                  for t in range(2 * NSPK)]
            for c in range(KCD):
                for d in range(2):
                    for s in range(NSPK):
                        dr_mm(ka[d * NSPK + s][:], wks, cts, c,
                              (d * P, (d + 1) * P), (s * SP, (s + 1) * SP),
                              c == 0, c == KCD - 1)
            for d in range(2):
                for s in range(NSPK):
                    nc.vector.tensor_copy(
                        kt[:, d * NK + s * SP: d * NK + (s + 1) * SP],
                        ka[d * NSPK + s][:])
            for jc in range(JC // 4):
                vp = kp.tile([P, HD], F32, tag="ka", name=f"vpk{jc}")
                for c in range(KCD):
                    dr_mm(vp[:], cts, wvs, c,
                          (jc * P, (jc + 1) * P), (0, HD),
                          c == 0, c == KCD - 1)
                dstv = vaug[:, jc * VW:(jc + 1) * VW].rearrange(
                    "p (h e) -> p h e", h=NH)
                nc.vector.tensor_scalar_mul(
                    dstv[:, :, 0:64],
                    vp[:].rearrange("p (h e) -> p h e", h=NH),
                    mkt[:, jc:jc + 1])
                nc.vector.tensor_scalar_mul(
                    dstv[:, :, 64:65], dstv[:, :, 64:65], mkt[:, jc:jc + 1])
            for s in range(ISP):
                q0 = kp.tile([P, SP], F32, tag="ka", name=f"q0_{s}")
                for c in range(KCD):
                    dr_mm(q0[:], wqs, xts, c,
                          (0, P), (s * SP, (s + 1) * SP),
                          c == 0, c == KCD - 1)
                nc.vector.tensor_copy(qt[:, s * SP:(s + 1) * SP], q0[:])

        # ---- attention-phase PSUM pools ----
        ps_pool = ctx.enter_context(
            tc.tile_pool(name="ps", bufs=2, space="PSUM"))
        po_pool = ctx.enter_context(
            tc.tile_pool(name="po", bufs=2, space="PSUM"))
        prp = ctx.enter_context(tc.tile_pool(name="pr", bufs=1, space="PSUM"))
        pr = prp.tile([P, ICH], F32, tag="pr")
        misc_pool = ctx.enter_context(
            tc.tile_pool(name="misc", bufs=1, space="PSUM"))

        # ---- filler emitters (PE work to hide under the exp pipeline) ----
        # Span tasks split into 2 pieces of KC/2 matmuls each so a filler
        # burst never outruns ACT's short exp pipeline buffer.
        def _pieces(alloc, mm, fin):
            st = {}
            cmid = max(1, KCD // 2)

            def p1():
                st["t"] = alloc()
                mm(st["t"], 0, cmid)

            def p2():
                mm(st["t"], cmid, KCD)
                fin(st["t"])
            return [p1, p2]

        def kproj_pieces(d, s0):
            def alloc():
                return misc_pool.tile([P, SP], F32, tag="m", name=f"kp{d}_{s0}")

            def mm(t, c0_, c1_):
                for c in range(c0_, c1_):
                    dr_mm(t[:], wks, cts, c,
                          (d * P, (d + 1) * P), (s0 * SP, (s0 + 1) * SP),
                          c == 0, c == KCD - 1)

            def fin(t):
                nc.vector.tensor_copy(
                    kt[:, d * NK + s0 * SP: d * NK + (s0 + 1) * SP], t[:])
            return _pieces(alloc, mm, fin)

        def qproj_pieces(d, ih, sq):
            c0 = ih * IHALF + sq * SP

            def alloc():
                return misc_pool.tile([P, SP], F32, tag="m",
                                      name=f"qp{d}_{ih}_{sq}")

            def mm(t, c0_, c1_):
                for c in range(c0_, c1_):
                    dr_mm(t[:], wqs, xts, c,
                          (d * P, (d + 1) * P), (c0, c0 + SP),
                          c == 0, c == KCD - 1)

            def fin(t):
                nc.vector.tensor_copy(
                    qt[:, d * NQ + c0: d * NQ + c0 + SP], t[:])
            return _pieces(alloc, mm, fin)

        def vproj_into(pool, tag, jc):
            """V projection for one j-chunk into vaug via a [P, HD] psum tile."""
            ps = pool.tile([P, HD], F32, tag=tag, name=f"vp{jc}")
            for c in range(KCD):
                dr_mm(ps[:], cts, wvs, c,
                      (jc * P, (jc + 1) * P), (0, HD),
                      c == 0, c == KCD - 1)
            dst = vaug[:, jc * VW:(jc + 1) * VW].rearrange(
                "p (h e) -> p h e", h=NH)
            nc.vector.tensor_scalar_mul(
                dst[:, :, 0:64],
                ps[:].rearrange("p (h e) -> p h e", h=NH),
                mkt[:, jc:jc + 1])
            nc.vector.tensor_scalar_mul(
                dst[:, :, 64:65], dst[:, :, 64:65], mkt[:, jc:jc + 1])

        def emit_oproj(i, tail=False):
            """Output projection for global i-chunk i."""
            if tail:
                # ACT is idle in the tail: stage copies there, DMA per span
                op = ps_pool.tile([P, OD], F32, tag="ps", name=f"op{i}")
                osb = osb_pool.tile([P, OD], F32, tag="osb")
                for o in range(OD // SP):
                    for d in range(DC):
                        nc.tensor.matmul(
                            op[:, o * SP:(o + 1) * SP],
                            ot_all[:, d * NQ + i * P: d * NQ + (i + 1) * P],
                            wo_t[:, d * OD + o * SP: d * OD + (o + 1) * SP],
                            start=(d == 0), stop=(d == DC - 1))
                    nc.scalar.copy(osb[:, o * SP:(o + 1) * SP],
                                   op[:, o * SP:(o + 1) * SP])
                    nc.sync.dma_start(
                        out_d.ap()[i * P:(i + 1) * P, o * SP:(o + 1) * SP],
                        osb[:, o * SP:(o + 1) * SP])
                return
            osb = osb_pool.tile([P, OD], F32, tag="osb")
            if True:
                for o in range(OD // SP):
                    op = misc_pool.tile([P, SP], F32, tag="m",
                                        name=f"op{i}_{o}")
                    for d in range(DC):
                        nc.tensor.matmul(
                            op[:],
                            ot_all[:, d * NQ + i * P: d * NQ + (i + 1) * P],
                            wo_t[:, d * OD + o * SP: d * OD + (o + 1) * SP],
                            start=(d == 0), stop=(d == DC - 1))
                    nc.vector.tensor_copy(osb[:, o * SP:(o + 1) * SP], op[:])
            nc.sync.dma_start(out_d.ap()[i * P:(i + 1) * P, :], osb[:])

        def oproj_span(i, o, osb):
            op = misc_pool.tile([P, SP], F32, tag="m", name=f"op{i}_{o}")
            for d in range(DC):
                nc.tensor.matmul(
                    op[:], ot_all[:, d * NQ + i * P: d * NQ + (i + 1) * P],
                    wo_t[:, d * OD + o * SP: d * OD + (o + 1) * SP],
                    start=(d == 0), stop=(d == DC - 1))
            nc.vector.tensor_copy(osb[:, o * SP:(o + 1) * SP], op[:])
            if o == OD // SP - 1:
                nc.sync.dma_start(out_d.ap()[i * P:(i + 1) * P, :], osb[:])

        # ---- attention units (flat QK stream, PV lags cross-unit) ----
        po_tiles = {}
        pts = {}
        norm_state = {}

        def emit_qk(u, jc):
            ih, h = u // NH, u % NH
            dc, hoff = h // 2, (h % 2) * 64
            ps = ps_pool.tile([P, IHALF], F32, tag="ps")
            for sq in range(ISP):
                i0 = ih * IHALF + sq * SP
                nc.tensor.matmul(
                    ps[:, sq * SP:(sq + 1) * SP],
                    kt[hoff:hoff + 64, dc * NK + jc * P: dc * NK + (jc + 1) * P],
                    qt[hoff:hoff + 64, dc * NQ + i0: dc * NQ + i0 + SP],
                    start=True, stop=True)
            pt = pts_pool.tile([P, IHALF], BF16, tag="pt", name=f"pt{u}_{jc}")
            nc.scalar.activation(pt[:], ps[:], AF.Exp,
                                 scale=SCALE / (WSCALE * WSCALE))
            pts[(u, jc)] = pt

        def emit_pv(u, jc):
            h = u % NH
            if jc == 0:
                po_tiles[u] = po_pool.tile([P, ICH * DH], F32, tag="po",
                                           name=f"po{u}")
            po = po_tiles[u]
            pt = pts[(u, jc)]
            for icl in range(ICH):
                # one accumulation group per PSUM bank: start/stop only on the
                # very first/last matmul touching the region (zero-region rule)
                st = (jc == 0 and icl == 0)
                sp = (jc == JC - 1 and icl == ICH - 1)
                nc.tensor.matmul(
                    po[:, icl * DH:(icl + 1) * DH],
                    pt[:, icl * P:(icl + 1) * P],
                    vaug[:, jc * VW + h * 65: jc * VW + h * 65 + 64],
                    start=st, stop=sp)
                nc.tensor.matmul(
                    pr[:, icl: icl + 1],
                    pt[:, icl * P:(icl + 1) * P],
                    vaug[:, jc * VW + h * 65 + 64: jc * VW + h * 65 + 65],
                    start=st, stop=sp)
            if jc >= 2:
                pts.pop((u, jc - 2), None)

        def emit_norm_a(u):
            """Per-partition reciprocal + normalize to bf16 staging."""
            po = po_tiles[u]
            rv = rv_pool.tile([P, ICH], F32, tag="rv")
            nc.vector.reciprocal(rv[:], pr[:, 0:ICH])
            on = on_pool.tile([P, ICH * DH], BF16, tag="on")
            for icl in range(ICH):
                nc.vector.tensor_scalar_mul(
                    on[:, icl * DH:(icl + 1) * DH],
                    po[:, icl * DH:(icl + 1) * DH], rv[:, icl:icl + 1])
            norm_state[u] = (on, gcur[0])

        def emit_norm_b(u):
            """PE transpose [i,d]->[d,i] and store to ot_all."""
            ih, h = u // NH, u % NH
            dc, hoff = h // 2, (h % 2) * 64
            on, _ = norm_state.pop(u)
            tr = misc_pool.tile([64, ICH * P], BF16, tag="m", name=f"tr{u}")
            for icl in range(ICH):
                nc.tensor.transpose(
                    tr[0:64, icl * P:(icl + 1) * P],
                    on[:, icl * DH:(icl + 1) * DH], ident[:])
            nc.vector.tensor_copy(
                ot_all[hoff:hoff + 64,
                       dc * NQ + ih * IHALF: dc * NQ + (ih + 1) * IHALF],
                tr[0:64, :])
            po_tiles.pop(u)
            if h == NH - 1:
                oproj_avail[0] = max(oproj_avail[0], (ih + 1) * ICH)

        # filler queue: (deadline_unit, emit_fn), pulled inside the jc loop
        # in span-sized pieces so ACT's 2-slot exp buffer never drains.
        fillers = []
        for sq in range(ISP):
            fillers.append((2, qproj_pieces(1, 0, sq)))
        for d in range(2, DC):
            for s0 in range(NSPK):
                fillers.append((2 * d, kproj_pieces(d, s0)))
            for sq in range(ISP):
                fillers.append((2 * d, qproj_pieces(d, 0, sq)))
        if NIH > 1:
            for d in range(DC):
                for sq in range(ISP):
                    fillers.append((NH + 2 * d, qproj_pieces(d, 1, sq)))
        fillers.reverse()  # pop() serves in original order
        microq = []
        ostart = (JC // 2 - 1, JC - 3) if NIH > 1 else ()
        ofin = (JC // 2 + 1, JC - 1) if NIH > 1 else ()

        osb_cur = [None]
        last_special = NIH > 1 and ICH % 2 == 0
        main_units = NUNITS - 1 if last_special else NUNITS
        steps = [(u, jc) for u in range(main_units) for jc in range(JC)]
        pv_i = [0]
        vp_done = [JC // 4]   # vaug chunks ready (built in kp phase)
        oproj_next = [0]
        oproj_avail = [0]

        def pv_ready(g):
            if pv_i[0] >= len(steps):
                return False
            pu, pj = steps[pv_i[0]]
            if pj >= vp_done[0] and vp_done[0] < JC:
                return False
            return pv_i[0] <= g - 3

        def drain_pv(g):
            while pv_ready(g):
                pu, pj = steps[pv_i[0]]
                emit_pv(pu, pj)
                pv_i[0] += 1
                if pj == JC - 1:
                    emit_norm_a(pu)

        nbh = (min(5, JC - 1), min(11, JC - 1))

        def emit_ready_norm_b(g):
            for v in sorted(norm_state):
                if norm_state[v][1] <= g - 2:
                    emit_norm_b(v)

        gcur = [0]
        for g, (u, jc) in enumerate(steps):
            gcur[0] = g
            emit_qk(u, jc)
            if jc % 2 == 1 and vp_done[0] < JC:
                vproj_into(ps_pool, "ps", vp_done[0])
                vp_done[0] += 1
            elif u > 0:
                if jc in nbh and norm_state:
                    emit_ready_norm_b(g)
                if (jc >= min(4, JC - 2) and jc % 2 == 0 and not microq
                        and fillers and (
                        fillers[-1][0] <= u + 2 or jc <= JC // 2)):
                    microq.extend(fillers.pop()[1])
                if microq and (jc % 2 == 0 or JC <= 8):
                    microq.pop(0)()
                if jc in ostart:
                    if (oproj_next[0] < oproj_avail[0]
                            and osb_cur[0] is None):
                        osb_cur[0] = osb_pool.tile(
                            [P, OD], F32, tag="osb", name=f"osbm{g}")
                        oproj_span(oproj_next[0], 0, osb_cur[0])
                elif jc in ofin and osb_cur[0] is not None:
                    oproj_span(oproj_next[0], OD // SP - 1, osb_cur[0])
                    oproj_next[0] += 1
                    osb_cur[0] = None
            drain_pv(g)

        if last_special:
            # Last unit runs as two 512-query sub-units so its first half's
            # output projection hides inside the second half's exp window.
            HQ = IHALF // 2
            HICH = ICH // 2
            L = NUNITS - 1
            lih, lh = L // NH, L % NH
            ldc, lhoff = lh // 2, (lh % 2) * 64
            sub_pts = {}
            sub_po = {}
            sub_on = {}
            sub_pvi = [0]
            sub_steps = [(iq, jc) for iq in range(2) for jc in range(JC)]

            def sub_qk(iq, jc):
                ps = ps_pool.tile([P, HQ], F32, tag="ps")
                i0 = lih * IHALF + iq * HQ
                nc.tensor.matmul(
                    ps[:],
                    kt[lhoff:lhoff + 64,
                       ldc * NK + jc * P: ldc * NK + (jc + 1) * P],
                    qt[lhoff:lhoff + 64, ldc * NQ + i0: ldc * NQ + i0 + HQ],
                    start=True, stop=True)
                pt = pts_pool.tile([P, HQ], BF16, tag="pt",
                                   name=f"spt{iq}_{jc}")
                nc.scalar.activation(pt[:], ps[:], AF.Exp,
                                     scale=SCALE / (WSCALE * WSCALE))
                sub_pts[(iq, jc)] = pt

            def sub_pv(iq, jc):
                if jc == 0:
                    sub_po[iq] = po_pool.tile([P, HICH * DH], F32, tag="po",
                                              name=f"spo{iq}")
                po = sub_po[iq]
                pt = sub_pts[(iq, jc)]
                for icl in range(HICH):
                    st = (jc == 0 and icl == 0)
                    sp = (jc == JC - 1 and icl == HICH - 1)
                    nc.tensor.matmul(
                        po[:, icl * DH:(icl + 1) * DH],
                        pt[:, icl * P:(icl + 1) * P],
                        vaug[:, jc * VW + lh * 65: jc * VW + lh * 65 + 64],
                        start=st, stop=sp)
                    nc.tensor.matmul(
                        pr[:, iq * HICH + icl: iq * HICH + icl + 1],
                        pt[:, icl * P:(icl + 1) * P],
                        vaug[:, jc * VW + lh * 65 + 64:
                             jc * VW + lh * 65 + 65],
                        start=st, stop=sp)

            def sub_norm_a(iq):
                po = sub_po[iq]
                rv = rv_pool.tile([P, HICH], F32, tag="rv")
                nc.vector.reciprocal(rv[:], pr[:, iq * HICH:(iq + 1) * HICH])
                on = on_pool.tile([P, HICH * DH], BF16, tag="on")
                for icl in range(HICH):
                    nc.vector.tensor_scalar_mul(
                        on[:, icl * DH:(icl + 1) * DH],
                        po[:, icl * DH:(icl + 1) * DH], rv[:, icl:icl + 1])
                sub_on[iq] = on

            def sub_norm_b(iq):
                on = sub_on.pop(iq)
                tr = misc_pool.tile([64, HICH * P], BF16, tag="m",
                                    name=f"str{iq}")
                for icl in range(HICH):
                    nc.tensor.transpose(
                        tr[0:64, icl * P:(icl + 1) * P],
                        on[:, icl * DH:(icl + 1) * DH], ident[:])
                i0 = lih * IHALF + iq * HQ
                nc.vector.tensor_copy(
                    ot_all[lhoff:lhoff + 64, ldc * NQ + i0: ldc * NQ + i0 + HQ],
                    tr[0:64, :])
                oproj_avail[0] = max(oproj_avail[0], (i0 + HQ) // P)

            def sub_drain(gg):
                while sub_pvi[0] < len(sub_steps) and sub_pvi[0] <= gg - 2:
                    siq, sjc = sub_steps[sub_pvi[0]]
                    sub_pv(siq, sjc)
                    sub_pvi[0] += 1
                    if sjc == JC - 1:
                        sub_norm_a(siq)

            for gg, (iq, jc) in enumerate(sub_steps):
                sub_qk(iq, jc)
                gcur[0] = len(steps) + gg
                drain_pv(len(steps) + gg)
                if iq == 0 and jc == 4 and norm_state:
                    emit_ready_norm_b(len(steps) + gg)
                if iq == 1:
                    if jc in (2, 3) and 0 in sub_on:
                        sub_norm_b(0)
                    if jc in (3, 7, 11):
                        if (oproj_next[0] < oproj_avail[0]
                                and osb_cur[0] is None):
                            osb_cur[0] = osb_pool.tile(
                                [P, OD], F32, tag="osb", name=f"osbs{gg}")
                            oproj_span(oproj_next[0], 0, osb_cur[0])
                    elif jc in (5, 9, 13) and osb_cur[0] is not None:
                        oproj_span(oproj_next[0], OD // SP - 1, osb_cur[0])
                        oproj_next[0] += 1
                        osb_cur[0] = None
                sub_drain(gg)
            gg = len(sub_steps)
            while sub_pvi[0] < len(sub_steps):
                sub_drain(gg)
                gg += 1
            for v in sorted(norm_state):
                emit_norm_b(v)
            if 1 in sub_on:
                sub_norm_b(1)
        else:
            g = len(steps)
            while pv_i[0] < len(steps):
                gcur[0] = g
                drain_pv(g)
                g += 1
            for v in sorted(norm_state):
                emit_norm_b(v)
        while fillers:
            for piece in fillers.pop()[1]:
                piece()
        while microq:
            microq.pop(0)()
        oproj_avail[0] = IC
        while oproj_next[0] < IC:
            emit_oproj(oproj_next[0], tail=True)
            oproj_next[0] += 1

    nc.compile()
    return nc


def fp8_hilo(a):
    """Split f32 array into compensated fp8e4m3 hi + lo parts."""
    f8 = ml_dtypes.float8_e4m3
    hi = a.astype(f8)
    lo = (a - hi.astype(np.float32)).astype(f8)
    return hi, lo


def shard_inputs(x, context, mask, Wq, Wk, Wv, Wo):
    """Host-side shard prep: fp8 hi/lo transposed activations + x64 weights."""
    bf = ml_dtypes.bfloat16
    in_maps = []
    for c in range(NCORES):
        b, hb = c // 2, c % 2
        cols = slice(hb * HD, (hb + 1) * HD)
        xth, xtl = fp8_hilo(np.ascontiguousarray(x[b].T))
        cth, ctl = fp8_hilo(np.ascontiguousarray(context[b].T))
        wqh, wql = fp8_hilo(np.ascontiguousarray(Wq[:, cols]) * WSCALE)
        wkh, wkl = fp8_hilo(np.ascontiguousarray(Wk[:, cols]) * WSCALE)
        wvh, wvl = fp8_hilo(np.ascontiguousarray(Wv[:, cols]) * WSCALE)
        in_maps.append({
            "xth": xth, "xtl": xtl, "cth": cth, "ctl": ctl,
            "wqh": wqh, "wql": wql, "wkh": wkh, "wkl": wkl,
            "wvh": wvh, "wvl": wvl,
            "wo": np.ascontiguousarray(Wo[cols, :]).astype(bf),
            "mk": mask[b].astype(np.float32),
        })
    return in_maps


_NC_CACHE = {}


def kernel(x, context, mask, Wq, Wk, Wv, Wo, bo, _trace=False):
    x = np.asarray(x, np.float32)
    context = np.asarray(context, np.float32)
    mask = np.asarray(mask)
    Wq, Wk, Wv = (np.asarray(a, np.float32) for a in (Wq, Wk, Wv))
    Wo, bo = np.asarray(Wo, np.float32), np.asarray(bo, np.float32)

    if "nc" not in _NC_CACHE:
        _NC_CACHE["nc"] = build_nc()
    nc = _NC_CACHE["nc"]

    in_maps = shard_inputs(x, context, mask, Wq, Wk, Wv, Wo)
    res = run_bass_kernel_spmd(nc, in_maps, list(range(NCORES)), trace=_trace)
    out = np.zeros((B, N, OD), np.float32)
    for c in range(NCORES):
        out[c // 2] += res.results[c]["out"]
    out += bo
    _NC_CACHE["last_res"] = res
    return out


# revision 38
# speedup vs baseline: 1.2647x; 1.0014x over previous
"""Trainium2 Bass kernel for nn_CrossAttention_7584912245418.

Sharding: batch*head-blocks across 8 cores. Core c handles batch b=c//2 and
head block hb=c%2 (8 of 16 heads). Weights column/row-sliced per head block;
no cross-device communication. Host feeds pre-transposed bf16 activations
(xT, cT); the two per-batch partial outputs (Wo row-split) + bias are summed
on host.

Per-core pipeline (cost-model-aware: matmul cost = out free size x cycle,
so PV is recast as tiny [i,d]-output matmuls; ACT exp ~255us is the other
near-critical engine and the schedule keeps it saturated):
  All K/Q/V projections run as compensated fp8e4m3 DoubleRow matmuls
  (hi*hi + hi*lo + lo*hi, weights prescaled x64 on host, descale folded
  into the exp scale and the vaug mask column) -> 0.5 cycles/row at
  ~bf16-or-better accuracy, DMA-neutral (fp8 hi+lo bytes == bf16 bytes).
  KT(d0,d1) = Wk_s^T @ cT   k-streamed against the ct DMA (8 PSUM banks)
  V(jc 0..3) projected in the same phase, hiding the xT DMA wait
  QT(d0, ihalf0) k-streamed against xT; rest of Q/K/V proj becomes filler
  units = (i-half, head), flat QK stream with cross-unit PV lag 3:
    per jc: S^T[j,i] = KT_chunk^T @ QT     (PSUM f32 [128, 1024])
            P^T = exp(SCALE*S^T)           (ACT, bf16, no max-sub)
            O[i,d] += P^T_chunk^T @ V      (65-row matmuls; rowsum via
            rowsum[i] += P^T_chunk^T @ mask col     one extra 1-row matmul)
    leftover K/Q projections ride as 4-matmul micro-pieces on even steps
    (never outrunning ACT's 2-exp pipeline buffer); V proj for jc 4..15
    rides odd steps of units 0-1.
    norm: per-partition recip+mul (no gpsimd broadcast), PE transpose
          (identity matmul) back to [d,i] bf16 ot_all.
  out = OT^T @ Wo per i-chunk; i-half 0's output projection hides inside
  i-half 1's exp window; the last unit runs as two 512-query sub-units so
  half its output projection hides inside the final exp window.
PSUM: ps 2x2 banks (QK/exp double buffer), po 2x1 (one accumulation group
per bank per unit), pr 1 (rowsums), misc 1 (fillers/oproj/transposes).
"""

import sys

for _p in ("/opt/trn_rl_repo",):
    if _p not in sys.path:
        sys.path.insert(0, _p)

from contextlib import ExitStack

import ml_dtypes
import numpy as np

import concourse.bass as bass
import concourse.mybir as mybir
import concourse.tile as tile
from concourse import bacc
from concourse.bass_utils import run_bass_kernel_spmd
from concourse.masks import make_identity

F32 = mybir.dt.float32
BF16 = mybir.dt.bfloat16
F8 = mybir.dt.float8e4
WSCALE = 64.0           # host scales W{q,k,v} by 64 into fp8 range
AF = mybir.ActivationFunctionType

# Full-problem constants
B, N, M = 4, 2048, 2048
QD, CD, OD = 1024, 1024, 1024
H, DH = 16, 64
SCALE = DH ** -0.5
NCORES = 8
NH = 8            # heads per core
HD = NH * DH      # 512, per-core inner dim
P = 128


def build_nc(NQ=N, NK=M, KD=QD, trace_sim=False):
    """Build the per-core SPMD program. NQ=query len, NK=kv len, KD=model dim."""
    KC = KD // P            # contraction chunks for projections
    JC = NK // P            # key-position chunks
    IC = NQ // P            # query-position chunks
    SP = 512                # projection matmul free-dim span
    IHALF = min(1024, NQ)   # queries per attention unit (and exp granularity)
    NIH = NQ // IHALF
    ISP = IHALF // SP       # spans per i-half
    ICH = IHALF // P        # i-chunks per half
    DC = HD // P            # 4 head-pair chunks (2 heads per chunk)
    VW = NH * 65            # vaug row width per j-chunk
    NUNITS = NIH * NH

    nc = bacc.Bacc("TRN2", target_bir_lowering=False, debug=False,
                   enable_asserts=False)

    xt_ds = [nc.dram_tensor(n, [KD, NQ], F8, kind="ExternalInput")
             for n in ("xth", "xtl")]
    ct_ds = [nc.dram_tensor(n, [KD, NK], F8, kind="ExternalInput")
             for n in ("cth", "ctl")]
    wq_ds = [nc.dram_tensor(n, [KD, HD], F8, kind="ExternalInput")
             for n in ("wqh", "wql")]
    wk_ds = [nc.dram_tensor(n, [KD, HD], F8, kind="ExternalInput")
             for n in ("wkh", "wkl")]
    wv_ds = [nc.dram_tensor(n, [KD, HD], F8, kind="ExternalInput")
             for n in ("wvh", "wvl")]
    wo_d = nc.dram_tensor("wo", [HD, OD], BF16, kind="ExternalInput")
    mk_d = nc.dram_tensor("mk", [NK], F32, kind="ExternalInput")
    out_d = nc.dram_tensor("out", [NQ, OD], F32, kind="ExternalOutput")
    KCD = KD // 256         # DoubleRow contraction chunks (256 rows each)
    DR = mybir.MatmulPerfMode.DoubleRow

    with tile.TileContext(nc, trace_sim=trace_sim) as tc, ExitStack() as ctx:
        # ---- persistent SBUF ----
        pp = ctx.enter_context(tc.tile_pool(name="persist", bufs=1))
        kt = pp.tile([P, DC * NK], BF16, tag="kt")
        qt = pp.tile([P, DC * NQ], BF16, tag="qt")
        vaug = pp.tile([P, JC * VW], BF16, tag="vaug")
        mkt = pp.tile([P, JC], F32, tag="mkt")
        ot_all = pp.tile([P, DC * NQ], BF16, tag="ot_all")
        ident = pp.tile([P, P], BF16, tag="ident")
        wo_t = pp.tile([P, DC * OD], BF16, tag="wo_t")
        def dr_tiles(nm, cols):
            # [hi/lo][chunk] tiles laid out [P, 2, cols] for DoubleRow
            return [[pp.tile([P, 2 * cols], F8, tag=f"{nm}{hl}{c}",
                             name=f"{nm}{hl}{c}") for c in range(KCD)]
                    for hl in range(2)]

        cts = dr_tiles("ct", NK)
        wks = dr_tiles("wk", HD)
        wvs = dr_tiles("wv", HD)
        xts = dr_tiles("xt", NQ)
        wqs = dr_tiles("wq", HD)

        def dr3(t, cols):
            return t[:].rearrange("p (t n) -> p t n", t=2)

        def dr_dma(tile, dram, c, c0, c1):
            nc.sync.dma_start(
                dr3(tile, 0)[:, :, c0:c1],
                dram.ap()[256 * c: 256 * (c + 1), c0:c1].rearrange(
                    "(t p) n -> p t n", t=2))

        pts_pool = ctx.enter_context(tc.tile_pool(name="pts", bufs=14))
        on_pool = ctx.enter_context(tc.tile_pool(name="on", bufs=2))
        rv_pool = ctx.enter_context(tc.tile_pool(name="rv", bufs=2))
        osb_pool = ctx.enter_context(tc.tile_pool(name="osb", bufs=3))

        # ---- input DMAs (order = DMA device order: K-proj inputs first) ----
        for c in range(KCD):
            for hl in range(2):
                dr_dma(wks[hl][c], wk_ds[hl], c, 0, HD)
                dr_dma(cts[hl][c], ct_ds[hl], c, 0, NK)
                dr_dma(wvs[hl][c], wv_ds[hl], c, 0, HD)
        nc.sync.dma_start(mkt[:], mk_d.ap().rearrange("(jc p) -> p jc", p=P))
        for c in range(KCD):
            for hl in range(2):
                dr_dma(wqs[hl][c], wq_ds[hl], c, 0, P)
                dr_dma(xts[hl][c], xt_ds[hl], c, 0, IHALF)
        for c in range(KCD):
            for hl in range(2):
                dr_dma(wqs[hl][c], wq_ds[hl], c, P, HD)
                if NQ > IHALF:
                    dr_dma(xts[hl][c], xt_ds[hl], c, IHALF, NQ)
        for d in range(DC):
            nc.sync.dma_start(wo_t[:, d * OD:(d + 1) * OD],
                              wo_d.ap()[d * P:(d + 1) * P, :])

        make_identity(nc, ident[:])
        warm = pp.tile([1, 1], BF16, tag="warm")
        nc.scalar.activation(warm[:], ident[0:1, 0:1], AF.Exp, scale=1.0)
        vz = vaug[:].rearrange("p (jc h e) -> p (jc h) e", jc=JC, h=NH)
        nc.vector.memset(vz[:, :, 64:65], WSCALE)

        # Each projection matmul = 3 compensated fp8 DoubleRow terms:
        # hi*hi + hi*lo + lo*hi (lo*lo dropped, ~1e-6 relative).
        TERMS = ((0, 0), (0, 1), (1, 0))

        def dr_mm(out, wt, xt_, c, wsl, xsl, start, stop):
            for ti, (wh, xh) in enumerate(TERMS):
                nc.tensor.matmul(
                    out, dr3(wt[wh][c], 0)[:, :, wsl[0]:wsl[1]],
                    dr3(xt_[xh][c], 0)[:, :, xsl[0]:xsl[1]],
                    start=(start and ti == 0), stop=(stop and ti == 2),
                    perf_mode=DR)

        # ---- K-proj d=0,1 k-streamed against the ct DMA; then Q(d0, ih0) ----
        NSPK = NK // SP
        with tc.tile_pool(name="kp", bufs=2 * NSPK, space="PSUM") as kp:
            ka = [kp.tile([P, SP], F32, tag="ka", name=f"ka{t}")
                  for t in range(2 * NSPK)]
            for c in range(KCD):
                for d in range(2):
                    for s in range(NSPK):
                        dr_mm(ka[d * NSPK + s][:], wks, cts, c,
                              (d * P, (d + 1) * P), (s * SP, (s + 1) * SP),
                              c == 0, c == KCD - 1)
            for d in range(2):
                for s in range(NSPK):
                    nc.vector.tensor_copy(
                        kt[:, d * NK + s * SP: d * NK + (s + 1) * SP],
                        ka[d * NSPK + s][:])
            for jc in range(JC // 4):
                vp = kp.tile([P, HD], F32, tag="ka", name=f"vpk{jc}")
                for c in range(KCD):
                    dr_mm(vp[:], cts, wvs, c,
                          (jc * P, (jc + 1) * P), (0, HD),
                          c == 0, c == KCD - 1)
                dstv = vaug[:, jc * VW:(jc + 1) * VW].rearrange(
                    "p (h e) -> p h e", h=NH)
                nc.vector.tensor_scalar_mul(
                    dstv[:, :, 0:64],
                    vp[:].rearrange("p (h e) -> p h e", h=NH),
                    mkt[:, jc:jc + 1])
                nc.vector.tensor_scalar_mul(
                    dstv[:, :, 64:65], dstv[:, :, 64:65], mkt[:, jc:jc + 1])
            for s in range(ISP):
                q0 = kp.tile([P, SP], F32, tag="ka", name=f"q0_{s}")
                for c in range(KCD):
                    dr_mm(q0[:], wqs, xts, c,
                          (0, P), (s * SP, (s + 1) * SP),
                          c == 0, c == KCD - 1)
                nc.vector.tensor_copy(qt[:, s * SP:(s + 1) * SP], q0[:])

        # ---- attention-phase PSUM pools ----
        ps_pool = ctx.enter_context(
            tc.tile_pool(name="ps", bufs=2, space="PSUM"))
        po_pool = ctx.enter_context(
            tc.tile_pool(name="po", bufs=2, space="PSUM"))
        prp = ctx.enter_context(tc.tile_pool(name="pr", bufs=1, space="PSUM"))
        pr = prp.tile([P, ICH], F32, tag="pr")
        misc_pool = ctx.enter_context(
            tc.tile_pool(name="misc", bufs=1, space="PSUM"))

        # ---- filler emitters (PE work to hide under the exp pipeline) ----
        # Span tasks split into 2 pieces of KC/2 matmuls each so a filler
        # burst never outruns ACT's short exp pipeline buffer.
        def _pieces(alloc, mm, fin):
            st = {}
            cmid = max(1, KCD // 2)

            def p1():
                st["t"] = alloc()
                mm(st["t"], 0, cmid)

            def p2():
                mm(st["t"], cmid, KCD)
                fin(st["t"])
            return [p1, p2]

        def kproj_pieces(d, s0):
            def alloc():
                return misc_pool.tile([P, SP], F32, tag="m", name=f"kp{d}_{s0}")

            def mm(t, c0_, c1_):
                for c in range(c0_, c1_):
                    dr_mm(t[:], wks, cts, c,
                          (d * P, (d + 1) * P), (s0 * SP, (s0 + 1) * SP),
                          c == 0, c == KCD - 1)

            def fin(t):
                nc.vector.tensor_copy(
                    kt[:, d * NK + s0 * SP: d * NK + (s0 + 1) * SP], t[:])
            return _pieces(alloc, mm, fin)

        def qproj_pieces(d, ih, sq):
            c0 = ih * IHALF + sq * SP

            def alloc():
                return misc_pool.tile([P, SP], F32, tag="m",
                                      name=f"qp{d}_{ih}_{sq}")

            def mm(t, c0_, c1_):
                for c in range(c0_, c1_):
                    dr_mm(t[:], wqs, xts, c,
                          (d * P, (d + 1) * P), (c0, c0 + SP),
                          c == 0, c == KCD - 1)

            def fin(t):
                nc.vector.tensor_copy(
                    qt[:, d * NQ + c0: d * NQ + c0 + SP], t[:])
            return _pieces(alloc, mm, fin)

        def vproj_into(pool, tag, jc):
            """V projection for one j-chunk into vaug via a [P, HD] psum tile."""
            ps = pool.tile([P, HD], F32, tag=tag, name=f"vp{jc}")
            for c in range(KCD):
                dr_mm(ps[:], cts, wvs, c,
                      (jc * P, (jc + 1) * P), (0, HD),
                      c == 0, c == KCD - 1)
            dst = vaug[:, jc * VW:(jc + 1) * VW].rearrange(
                "p (h e) -> p h e", h=NH)
            nc.vector.tensor_scalar_mul(
                dst[:, :, 0:64],
                ps[:].rearrange("p (h e) -> p h e", h=NH),
                mkt[:, jc:jc + 1])
            nc.vector.tensor_scalar_mul(
                dst[:, :, 64:65], dst[:, :, 64:65], mkt[:, jc:jc + 1])

        def emit_oproj(i, tail=False):
            """Output projection for global i-chunk i."""
            if tail:
                osb = osb_pool.tile([P, OD], F32, tag="osb")
                for o in range(OD // SP):
                    op = ps_pool.tile([P, SP], F32, tag="ps",
                                      name=f"op{i}_{o}")
                    for d in range(DC):
                        nc.tensor.matmul(
                            op[:],
                            ot_all[:, d * NQ + i * P: d * NQ + (i + 1) * P],
                            wo_t[:, d * OD + o * SP: d * OD + (o + 1) * SP],
                            start=(d == 0), stop=(d == DC - 1))
                    nc.vector.tensor_copy(osb[:, o * SP:(o + 1) * SP], op[:])
                    if i == IC - 1:
                        nc.sync.dma_start(
                            out_d.ap()[i * P:(i + 1) * P,
                                       o * SP:(o + 1) * SP],
                            osb[:, o * SP:(o + 1) * SP])
                if i != IC - 1:
                    nc.sync.dma_start(out_d.ap()[i * P:(i + 1) * P, :],
                                      osb[:])
                return
            osb = osb_pool.tile([P, OD], F32, tag="osb")
            if True:
                for o in range(OD // SP):
                    op = misc_pool.tile([P, SP], F32, tag="m",
                                        name=f"op{i}_{o}")
                    for d in range(DC):
                        nc.tensor.matmul(
                            op[:],
                            ot_all[:, d * NQ + i * P: d * NQ + (i + 1) * P],
                            wo_t[:, d * OD + o * SP: d * OD + (o + 1) * SP],
                            start=(d == 0), stop=(d == DC - 1))
                    nc.vector.tensor_copy(osb[:, o * SP:(o + 1) * SP], op[:])
            nc.sync.dma_start(out_d.ap()[i * P:(i + 1) * P, :], osb[:])

        def oproj_span(i, o, osb):
            op = misc_pool.tile([P, SP], F32, tag="m", name=f"op{i}_{o}")
            for d in range(DC):
                nc.tensor.matmul(
                    op[:], ot_all[:, d * NQ + i * P: d * NQ + (i + 1) * P],
                    wo_t[:, d * OD + o * SP: d * OD + (o + 1) * SP],
                    start=(d == 0), stop=(d == DC - 1))
            nc.vector.tensor_copy(osb[:, o * SP:(o + 1) * SP], op[:])
            if o == OD // SP - 1:
                nc.sync.dma_start(out_d.ap()[i * P:(i + 1) * P, :], osb[:])

        # ---- attention units (flat QK stream, PV lags cross-unit) ----
        po_tiles = {}
        pts = {}
        norm_state = {}

        def emit_qk(u, jc):
            ih, h = u // NH, u % NH
            dc, hoff = h // 2, (h % 2) * 64
            ps = ps_pool.tile([P, IHALF], F32, tag="ps")
            for sq in range(ISP):
                i0 = ih * IHALF + sq * SP
                nc.tensor.matmul(
                    ps[:, sq * SP:(sq + 1) * SP],
                    kt[hoff:hoff + 64, dc * NK + jc * P: dc * NK + (jc + 1) * P],
                    qt[hoff:hoff + 64, dc * NQ + i0: dc * NQ + i0 + SP],
                    start=True, stop=True)
            pt = pts_pool.tile([P, IHALF], BF16, tag="pt", name=f"pt{u}_{jc}")
            nc.scalar.activation(pt[:], ps[:], AF.Exp,
                                 scale=SCALE / (WSCALE * WSCALE))
            pts[(u, jc)] = pt

        def emit_pv(u, jc):
            h = u % NH
            if jc == 0:
                po_tiles[u] = po_pool.tile([P, ICH * DH], F32, tag="po",
                                           name=f"po{u}")
            po = po_tiles[u]
            pt = pts[(u, jc)]
            for icl in range(ICH):
                # one accumulation group per PSUM bank: start/stop only on the
                # very first/last matmul touching the region (zero-region rule)
                st = (jc == 0 and icl == 0)
                sp = (jc == JC - 1 and icl == ICH - 1)
                nc.tensor.matmul(
                    po[:, icl * DH:(icl + 1) * DH],
                    pt[:, icl * P:(icl + 1) * P],
                    vaug[:, jc * VW + h * 65: jc * VW + h * 65 + 64],
                    start=st, stop=sp)
                nc.tensor.matmul(
                    pr[:, icl: icl + 1],
                    pt[:, icl * P:(icl + 1) * P],
                    vaug[:, jc * VW + h * 65 + 64: jc * VW + h * 65 + 65],
                    start=st, stop=sp)
            if jc >= 2:
                pts.pop((u, jc - 2), None)

        def emit_norm_a(u):
            """Per-partition reciprocal + normalize to bf16 staging."""
            po = po_tiles[u]
            rv = rv_pool.tile([P, ICH], F32, tag="rv")
            nc.vector.reciprocal(rv[:], pr[:, 0:ICH])
            on = on_pool.tile([P, ICH * DH], BF16, tag="on")
            for icl in range(ICH):
                nc.vector.tensor_scalar_mul(
                    on[:, icl * DH:(icl + 1) * DH],
                    po[:, icl * DH:(icl + 1) * DH], rv[:, icl:icl + 1])
            norm_state[u] = (on, gcur[0])

        def emit_norm_b(u):
            """PE transpose [i,d]->[d,i] and store to ot_all."""
            ih, h = u // NH, u % NH
            dc, hoff = h // 2, (h % 2) * 64
            on, _ = norm_state.pop(u)
            tr = misc_pool.tile([64, ICH * P], BF16, tag="m", name=f"tr{u}")
            for icl in range(ICH):
                nc.tensor.transpose(
                    tr[0:64, icl * P:(icl + 1) * P],
                    on[:, icl * DH:(icl + 1) * DH], ident[:])
            nc.vector.tensor_copy(
                ot_all[hoff:hoff + 64,
                       dc * NQ + ih * IHALF: dc * NQ + (ih + 1) * IHALF],
                tr[0:64, :])
            po_tiles.pop(u)
            if h == NH - 1:
                oproj_avail[0] = max(oproj_avail[0], (ih + 1) * ICH)

        # filler queue: (deadline_unit, emit_fn), pulled inside the jc loop
        # in span-sized pieces so ACT's 2-slot exp buffer never drains.
        fillers = []
        for sq in range(ISP):
            fillers.append((2, qproj_pieces(1, 0, sq)))
        for d in range(2, DC):
            for s0 in range(NSPK):
                fillers.append((2 * d, kproj_pieces(d, s0)))
            for sq in range(ISP):
                fillers.append((2 * d, qproj_pieces(d, 0, sq)))
        if NIH > 1:
            for d in range(DC):
                for sq in range(ISP):
                    fillers.append((NH + 2 * d, qproj_pieces(d, 1, sq)))
        fillers.reverse()  # pop() serves in original order
        microq = []
        ostart = (JC // 2 - 1, JC - 3) if NIH > 1 else ()
        ofin = (JC // 2 + 1, JC - 1) if NIH > 1 else ()

        osb_cur = [None]
        last_special = NIH > 1 and ICH % 2 == 0
        main_units = NUNITS - 1 if last_special else NUNITS
        steps = [(u, jc) for u in range(main_units) for jc in range(JC)]
        pv_i = [0]
        vp_done = [JC // 4]   # vaug chunks ready (built in kp phase)
        oproj_next = [0]
        oproj_avail = [0]

        def pv_ready(g):
            if pv_i[0] >= len(steps):
                return False
            pu, pj = steps[pv_i[0]]
            if pj >= vp_done[0] and vp_done[0] < JC:
                return False
            return pv_i[0] <= g - 3

        def drain_pv(g):
            while pv_ready(g):
                pu, pj = steps[pv_i[0]]
                emit_pv(pu, pj)
                pv_i[0] += 1
                if pj == JC - 1:
                    emit_norm_a(pu)

        nbh = (min(5, JC - 1), min(11, JC - 1))

        def emit_ready_norm_b(g):
            for v in sorted(norm_state):
                if norm_state[v][1] <= g - 2:
                    emit_norm_b(v)

        gcur = [0]
        for g, (u, jc) in enumerate(steps):
            gcur[0] = g
            emit_qk(u, jc)
            if jc % 2 == 1 and vp_done[0] < JC:
                vproj_into(ps_pool, "ps", vp_done[0])
                vp_done[0] += 1
            elif u > 0:
                if jc in nbh and norm_state:
                    emit_ready_norm_b(g)
                if (jc >= min(4, JC - 2) and jc % 2 == 0 and not microq
                        and fillers and (
                        fillers[-1][0] <= u + 2 or jc <= JC // 2)):
                    microq.extend(fillers.pop()[1])
                if microq and (jc % 2 == 0 or JC <= 8):
                    microq.pop(0)()
                if jc in ostart:
                    if (oproj_next[0] < oproj_avail[0]
                            and osb_cur[0] is None):
                        osb_cur[0] = osb_pool.tile(
                            [P, OD], F32, tag="osb", name=f"osbm{g}")
                        oproj_span(oproj_next[0], 0, osb_cur[0])
                elif jc in ofin and osb_cur[0] is not None:
                    oproj_span(oproj_next[0], OD // SP - 1, osb_cur[0])
                    oproj_next[0] += 1
                    osb_cur[0] = None
            drain_pv(g)

        if last_special:
            # Last unit runs as two 512-query sub-units so its first half's
            # output projection hides inside the second half's exp window.
            HQ = IHALF // 2
            HICH = ICH // 2
            L = NUNITS - 1
            lih, lh = L // NH, L % NH
            ldc, lhoff = lh // 2, (lh % 2) * 64
            sub_pts = {}
            sub_po = {}
            sub_on = {}
            sub_pvi = [0]
            sub_steps = [(iq, jc) for iq in range(2) for jc in range(JC)]

            def sub_qk(iq, jc):
                ps = ps_pool.tile([P, HQ], F32, tag="ps")
                i0 = lih * IHALF + iq * HQ
                nc.tensor.matmul(
                    ps[:],
                    kt[lhoff:lhoff + 64,
                       ldc * NK + jc * P: ldc * NK + (jc + 1) * P],
                    qt[lhoff:lhoff + 64, ldc * NQ + i0: ldc * NQ + i0 + HQ],
                    start=True, stop=True)
                pt = pts_pool.tile([P, HQ], BF16, tag="pt",
                                   name=f"spt{iq}_{jc}")
                nc.scalar.activation(pt[:], ps[:], AF.Exp,
                                     scale=SCALE / (WSCALE * WSCALE))
                sub_pts[(iq, jc)] = pt

            def sub_pv(iq, jc):
                if jc == 0:
                    sub_po[iq] = po_pool.tile([P, HICH * DH], F32, tag="po",
                                              name=f"spo{iq}")
                po = sub_po[iq]
                pt = sub_pts[(iq, jc)]
                for icl in range(HICH):
                    st = (jc == 0 and icl == 0)
                    sp = (jc == JC - 1 and icl == HICH - 1)
                    nc.tensor.matmul(
                        po[:, icl * DH:(icl + 1) * DH],
                        pt[:, icl * P:(icl + 1) * P],
                        vaug[:, jc * VW + lh * 65: jc * VW + lh * 65 + 64],
                        start=st, stop=sp)
                    nc.tensor.matmul(
                        pr[:, iq * HICH + icl: iq * HICH + icl + 1],
                        pt[:, icl * P:(icl + 1) * P],
                        vaug[:, jc * VW + lh * 65 + 64:
                             jc * VW + lh * 65 + 65],
                        start=st, stop=sp)

            def sub_norm_a(iq):
                po = sub_po[iq]
                rv = rv_pool.tile([P, HICH], F32, tag="rv")
                nc.vector.reciprocal(rv[:], pr[:, iq * HICH:(iq + 1) * HICH])
                on = on_pool.tile([P, HICH * DH], BF16, tag="on")
                for icl in range(HICH):
                    nc.vector.tensor_scalar_mul(
                        on[:, icl * DH:(icl + 1) * DH],
                        po[:, icl * DH:(icl + 1) * DH], rv[:, icl:icl + 1])
                sub_on[iq] = on

            def sub_norm_b(iq):
                on = sub_on.pop(iq)
                tr = misc_pool.tile([64, HICH * P], BF16, tag="m",
                                    name=f"str{iq}")
                for icl in range(HICH):
                    nc.tensor.transpose(
                        tr[0:64, icl * P:(icl + 1) * P],
                        on[:, icl * DH:(icl + 1) * DH], ident[:])
                i0 = lih * IHALF + iq * HQ
                nc.vector.tensor_copy(
                    ot_all[lhoff:lhoff + 64, ldc * NQ + i0: ldc * NQ + i0 + HQ],
                    tr[0:64, :])
                oproj_avail[0] = max(oproj_avail[0], (i0 + HQ) // P)

            def sub_drain(gg):
                while sub_pvi[0] < len(sub_steps) and sub_pvi[0] <= gg - 2:
                    siq, sjc = sub_steps[sub_pvi[0]]
                    sub_pv(siq, sjc)
                    sub_pvi[0] += 1
                    if sjc == JC - 1:
                        sub_norm_a(siq)

            for gg, (iq, jc) in enumerate(sub_steps):
                sub_qk(iq, jc)
                gcur[0] = len(steps) + gg
                drain_pv(len(steps) + gg)
                if iq == 0 and jc == 4 and norm_state:
                    emit_ready_norm_b(len(steps) + gg)
                if iq == 1:
                    if jc in (2, 3) and 0 in sub_on:
                        sub_norm_b(0)
                    if jc in (3, 7, 11):
                        if (oproj_next[0] < oproj_avail[0]
                                and osb_cur[0] is None):
                            osb_cur[0] = osb_pool.tile(
                                [P, OD], F32, tag="osb", name=f"osbs{gg}")
                            oproj_span(oproj_next[0], 0, osb_cur[0])
                    elif jc in (5, 9, 13) and osb_cur[0] is not None:
                        oproj_span(oproj_next[0], OD // SP - 1, osb_cur[0])
                        oproj_next[0] += 1
                        osb_cur[0] = None
                sub_drain(gg)
            gg = len(sub_steps)
            while sub_pvi[0] < len(sub_steps):
                sub_drain(gg)
                gg += 1
            for v in sorted(norm_state):
                emit_norm_b(v)
            if 1 in sub_on:
                sub_norm_b(1)
        else:
            g = len(steps)
            while pv_i[0] < len(steps):
                gcur[0] = g
                drain_pv(g)
                g += 1
            for v in sorted(norm_state):
                emit_norm_b(v)
        while fillers:
            for piece in fillers.pop()[1]:
                piece()
        while microq:
            microq.pop(0)()
        oproj_avail[0] = IC
        while oproj_next[0] < IC:
            emit_oproj(oproj_next[0], tail=True)
            oproj_next[0] += 1

    nc.compile()
    return nc


def fp8_hilo(a):
    """Split f32 array into compensated fp8e4m3 hi + lo parts."""
    f8 = ml_dtypes.float8_e4m3
    hi = a.astype(f8)
    lo = (a - hi.astype(np.float32)).astype(f8)
    return hi, lo


def shard_inputs(x, context, mask, Wq, Wk, Wv, Wo):
    """Host-side shard prep: fp8 hi/lo transposed activations + x64 weights."""
    bf = ml_dtypes.bfloat16
    in_maps = []
    for c in range(NCORES):
        b, hb = c // 2, c % 2
        cols = slice(hb * HD, (hb + 1) * HD)
        xth, xtl = fp8_hilo(np.ascontiguousarray(x[b].T))
        cth, ctl = fp8_hilo(np.ascontiguousarray(context[b].T))
        wqh, wql = fp8_hilo(np.ascontiguousarray(Wq[:, cols]) * WSCALE)
        wkh, wkl = fp8_hilo(np.ascontiguousarray(Wk[:, cols]) * WSCALE)
        wvh, wvl = fp8_hilo(np.ascontiguousarray(Wv[:, cols]) * WSCALE)
        in_maps.append({
            "xth": xth, "xtl": xtl, "cth": cth, "ctl": ctl,
            "wqh": wqh, "wql": wql, "wkh": wkh, "wkl": wkl,
            "wvh": wvh, "wvl": wvl,
            "wo": np.ascontiguousarray(Wo[cols, :]).astype(bf),
            "mk": mask[b].astype(np.float32),
        })
    return in_maps


_NC_CACHE = {}


def kernel(x, context, mask, Wq, Wk, Wv, Wo, bo, _trace=False):
    x = np.asarray(x, np.float32)
    context = np.asarray(context, np.float32)
    mask = np.asarray(mask)
    Wq, Wk, Wv = (np.asarray(a, np.float32) for a in (Wq, Wk, Wv))
    Wo, bo = np.asarray(Wo, np.float32), np.asarray(bo, np.float32)

    if "nc" not in _NC_CACHE:
        _NC_CACHE["nc"] = build_nc()
    nc = _NC_CACHE["nc"]

    in_maps = shard_inputs(x, context, mask, Wq, Wk, Wv, Wo)
    res = run_bass_kernel_spmd(nc, in_maps, list(range(NCORES)), trace=_trace)
    out = np.zeros((B, N, OD), np.float32)
    for c in range(NCORES):
        out[c // 2] += res.results[c]["out"]
    out += bo
    _NC_CACHE["last_res"] = res
    return out
